# revision 1
# baseline (speedup 1.0000x reference)
"""DSALite sparse-attention Trainium2 kernel.

Problem: B=1, nH=4, T=4096, hd=128 attention where the mask is derived from
8x-downsampled scores: per full row, threshold = 128th largest of the 512
downsampled (and u-scaled) scores, mask = sigmoid((s - thr) * 10 * u) * causal,
scores += (1-mask) * (-1e9), softmax, @V.

Sharding: 8 cores = 4 heads x 2 row-parities.  Core (h, r) handles head h and
query blocks {128k : k % 2 == r} (interleaved 128-row blocks, which balances
the causal work across cores).  Two Bass programs are built (one per parity,
since the causal structure is baked into static column ranges) and dispatched
concurrently on devices 0-3 (parity 0) and 4-7 (parity 1).

Structure per core (Tile-scheduled, ~2.3k instructions):
  - K^T / Q^T via PE transposes; exact-f32 downsampled-score matmul; exact
    per-row 128th-largest threshold via DVE max8 + match_replace.
  - Per 128-row block: replicate ds rows via 0/1 matmul, sigmoid via
    exp + correctly-rounded DVE reciprocal, F = fl(fl(1-m)*(-1e9)),
    A = min(F, ds-causal); fp32r QK^T restricted to the causal prefix;
    X = fl(fl(S*scale) + A) on DVE (diagonal block fixed elementwise via
    copy_predicated); ACT exp with accumulated row-sum.
  - Suffix (fully-masked) columns are never materialized: their uniform
    value C = exp(-1e9 - M) contributes C * (suffix V column-sum) via K=1
    rank-1 matmuls injected into the PV PSUM accumulator, and C * n_suffix
    is added to the softmax denominator.
  - PV: PE-transpose P chunks, fp32r V^T @ P^T accumulation (512-wide),
    transpose back, scale by 1/den, one output DMA per block.

Numerical-fidelity notes (the mask path is bucket-quantized by the f32
rounding of (1-m)*(-1e9), so it is replicated op-for-op):
  - downsampled scores use exact fp32 matmuls; the big QK^T / PV matmuls use
    fp32r (1 cyc/row at N=512, ~2^-21 relative input rounding, smooth path
    only -- S-noise there provably cannot flip mask buckets).
  - sigmoid is computed as 1/(1 + exp(-arg)) with the HW's correctly-rounded
    DVE reciprocal: the ACT Sigmoid *table* is 1 ulp low at the
    saturation-to-1.0 cliff, which flips (1-m)*(-1e9) between 0 and -59.6
    and corrupts ~half the rows; the exp+reciprocal form reproduces XLA's
    f32 saturation exactly.
  - softmax subtracts M = max over allowed F (within +-|S| of the true row
    max, which only shifts num/den by a common per-row factor).

Measured: hardware absmax error 0.0104 vs the f32 reference (rel 2.7e-3,
1 row > 1e-2 of 16384); TimelineSim cost-model estimate ~201/204 us for the
two programs (run concurrently).  The block partition KBS[] balances the
critical path between the programs, GORDERS[] gives each parity program its
own group processing order (their causal block mixes prefer different
tails), and the last two groups' PV is emitted in two halves so it
overlaps those blocks' scores/exp.

Schedule-head optimization (-11.5us vs the ~217/220us first cut): the
per-row top-128 threshold search (31 serial DVE max8/match_replace rounds
per sds tile) is the head of the whole schedule and needs only the
8x-downsampled K/Q rows.  Those 1/8 subsets are loaded FIRST via dedicated
strided DMAs and transposed immediately, so sds + top-k start at ~12us
instead of ~29us; the full K^T/Q^T prep and the large V load stream
afterwards, overlapped with the top-k rounds (V is not consumed until the
suffix column-sums / first PV).  The tile-1 search itself (serving query
blocks 8-15) is deferred until after group 0's emission so it overlaps
group 0's PE/ACT-heavy PV phase instead of blocking the group-0 mask
chains in the in-order DVE queue.
"""

import os
import numpy as np

import concourse.bass as bass
import concourse.bacc as bacc
import concourse.mybir as mybir
import concourse.tile as tile
from concourse.masks import make_identity

F32 = mybir.dt.float32
F32R = mybir.dt.float32r
ALU = mybir.AluOpType
ACTF = mybir.ActivationFunctionType
AX = mybir.AxisListType

B, NH, T, HD = 1, 4, 4096, 128
STRIDE = 8
TDS = T // STRIDE          # 512 downsampled positions
KDS = 128                  # 128th largest (k=1024 over 8x-repeated row)
NEG = -1e9
SCALE = HD ** -0.5
ZAP = -1e30                # match_replace fill, far below any score

NB = 16                    # 128-row query blocks per core
QPB = 128
NLQ = NB * QPB             # 2048 local query rows
NG = 4                     # PV groups of 4 blocks (512 q)
CH = T // 128              # 32 key/value chunks
CMW = 1008                 # sliding ds-causal const width
GORDERS = {0: [0, 1, 2, 3], 1: [1, 0, 3, 2]}      # group processing order (tunable)
# absolute 128-row block indices per program; near-equal causal work
KBS = [
    [0, 2, 4, 6, 8, 10, 12, 14, 18, 20, 22, 23, 24, 27, 29, 31],
    [1, 3, 5, 7, 9, 11, 13, 15, 16, 17, 19, 21, 25, 26, 28, 30],
]


def _consts(nc, pool):
    """Build the constant tiles (identity, diag causal, ds-causal, Rep16)."""
    ident = pool.tile([128, 128], F32, tag="ident")
    make_identity(nc, ident[:])

    # c01inv[i, j] = 1 where j > i (strictly above diagonal) else 0
    c01inv = pool.tile([128, 128], mybir.dt.int8, tag="c01inv")
    nc.gpsimd.memset(c01inv[:], 1)
    nc.gpsimd.affine_select(
        out=c01inv[:], in_=c01inv[:], pattern=[[1, 128]], base=0,
        channel_multiplier=-1, compare_op=ALU.is_gt, fill=0,
    )

    # cm3[i, jj] = 0.0 if jj <= 496 + i//8 else -1e9 (sliding ds-causal mask:
    # block with absolute index kb reads cols [496-16*kb, 496-16*kb+512)).
    # jj <= 496 + i//8  <=>  -8*jj + 3968 + i >= 0  (integer jj, i)
    cm3 = pool.tile([128, CMW], F32, tag="cm3")
    nc.gpsimd.memset(cm3[:], 0.0)
    nc.gpsimd.affine_select(
        out=cm3[:], in_=cm3[:], pattern=[[-8, CMW]], base=3968,
        channel_multiplier=1, compare_op=ALU.is_ge, fill=NEG,
    )

    # rep128[bp][rr, i] = 1.0 iff rr == 16*bp + i//8 : replicates the 16 ds
    # rows at partitions [16bp, 16bp+16) onto 128 rows (matmul lhsT).
    reps = []
    for bp in range(8):
        rep = pool.tile([128, 128], F32, tag=f"rep{bp}")
        nc.gpsimd.memset(rep[:], 1.0)
        # keep where (i - 8*rr + 128*bp) >= 0, else 0
        nc.gpsimd.affine_select(
            out=rep[:], in_=rep[:], pattern=[[1, 128]], base=128 * bp,
            channel_multiplier=-8, compare_op=ALU.is_ge, fill=0.0)
        # keep where (-i + 8*rr - 128*bp + 7) >= 0, else 0
        nc.gpsimd.affine_select(
            out=rep[:], in_=rep[:], pattern=[[-1, 128]], base=7 - 128 * bp,
            channel_multiplier=8, compare_op=ALU.is_ge, fill=0.0)
        reps.append(rep)

    identr = pool.tile([128, 128], F32R, tag="identr")
    nc.vector.tensor_copy(identr[:], ident[:])

    negc = pool.tile([128, 1], F32, tag="negc")
    nc.gpsimd.memset(negc[:], NEG)

    return ident, identr, c01inv, cm3, reps, negc


DEBUG = False


def _kernel_body(tc, r, Qd, Kd, Vd, Ud, Od, dbg=None):
    nc = tc.nc
    from contextlib import ExitStack
    with ExitStack() as ctx:
        cpool = ctx.enter_context(tc.tile_pool(name="consts", bufs=1))
        inpool = ctx.enter_context(tc.tile_pool(name="inputs", bufs=1))
        ps2pool = ctx.enter_context(tc.tile_pool(name="ps2", bufs=3, space="PSUM"))
        swppool = ctx.enter_context(tc.tile_pool(name="swp", bufs=1, space="PSUM"))
        ps1pool = ctx.enter_context(tc.tile_pool(name="ps1", bufs=1, space="PSUM"))
        ptpool = ctx.enter_context(tc.tile_pool(name="ptps", bufs=2, space="PSUM"))
        outtpool = ctx.enter_context(tc.tile_pool(name="outtps", bufs=1, space="PSUM"))
        sdspool = ctx.enter_context(tc.tile_pool(name="sds", bufs=1))
        scrpool = ctx.enter_context(tc.tile_pool(name="scr", bufs=1))
        epool = ctx.enter_context(tc.tile_pool(name="e", bufs=2))
        fpool = ctx.enter_context(tc.tile_pool(name="f", bufs=2))
        aminpool = ctx.enter_context(tc.tile_pool(name="amin", bufs=4))
        smallpool = ctx.enter_context(tc.tile_pool(name="small", bufs=NB))
        tinypool = ctx.enter_context(tc.tile_pool(name="tiny", bufs=2))
        ppool = ctx.enter_context(tc.tile_pool(name="p", bufs=5))
        ptsbpool = ctx.enter_context(tc.tile_pool(name="ptsb", bufs=3))
        outtsbpool = ctx.enter_context(tc.tile_pool(name="outtsb", bufs=2))
        tmppool = ctx.enter_context(tc.tile_pool(name="tmp", bufs=2))
        swsbpool = ctx.enter_context(tc.tile_pool(name="swsb", bufs=1))

        ident, identr, c01inv, cm3, reps, negc = _consts(nc, cpool)

        # ---- load inputs ----
        # The per-row top-128 threshold search is the serial head of the
        # whole schedule, and it needs only the 8x-downsampled K/Q rows.
        # Load those 1/8 subsets FIRST via strided DMAs, run sds + top-k,
        # and only then stream the full K^T/Q^T and the V tile.
        kt = inpool.tile([128, T], F32R, tag="kt")    # K^T  [d, t]
        qt = inpool.tile([128, NLQ], F32R, tag="qt")  # Q^T  [d, q_local]
        kdst = inpool.tile([128, TDS], F32, tag="kdst")
        qdst = inpool.tile([128, TDS // 2], F32, tag="qdst")
        prep = ctx.enter_context(tc.tile_pool(name="prep", bufs=2))
        for dsrc, dsdst, nds in ((Kd, kdst, TDS), (Qd, qdst, TDS // 2)):
            natd = prep.tile([128, nds], F32, tag="natd", name="natd")
            nc.sync.dma_start(
                out=natd[:].rearrange("p (c d) -> p c d", d=128),
                in_=dsrc.rearrange("(c p s) d -> p c s d", p=128,
                                   s=STRIDE)[:, :, 0, :],
            )
            n3 = natd[:].rearrange("p (c d) -> p c d", d=128)
            ptd = ps2pool.tile([128, 512], F32, tag="ps2", name="ppsd")
            for j in range(nds // 128):
                nc.tensor.transpose(
                    ptd[:, 128 * j:128 * j + 128], n3[:, j, :], ident[:])
            nc.vector.tensor_copy(dsdst[:], ptd[:, 0:nds])

        ub = inpool.tile([128, NB], F32, tag="ub")
        nc.sync.dma_start(out=ub[:], in_=Ud[:])
        usc = inpool.tile([128, NB], F32, tag="usc")
        # u_scale = 1 + clip(U, 0, 1) ; clip = min(max(u, 0), 1)
        nc.vector.tensor_scalar(usc[:], ub[:], 0.0, 1.0, op0=ALU.max,
                                op1=ALU.min)
        nc.vector.tensor_scalar(usc[:], usc[:], 1.0, None, op0=ALU.add)
        vsb = inpool.tile([128, T], F32R, tag="vsb")  # [t_local, c, d] natural

        def full_prep():
            for src, dst, npieces in ((Kd, kt, 8), (Qd, qt, 4)):
                s3 = src.rearrange("(c p) d -> p c d", p=128)
                for c4 in range(npieces):
                    nat = prep.tile([128, 512], F32, tag="nat")
                    nc.sync.dma_start(
                        out=nat[:].rearrange("p (c d) -> p c d", d=128),
                        in_=s3[:, 4 * c4:4 * c4 + 4, :],
                    )
                    n3 = nat[:].rearrange("p (c d) -> p c d", d=128)
                    pt = ps2pool.tile([128, 512], F32, tag="ps2")
                    for j in range(4):
                        nc.tensor.transpose(
                            pt[:, 128 * j:128 * j + 128], n3[:, j, :],
                            ident[:])
                    nc.any.tensor_copy(dst[:, 512 * c4:512 * c4 + 512], pt[:])
            nc.sync.dma_start(
                out=vsb[:].rearrange("p (c d) -> p c d", d=128),
                in_=Vd.rearrange("(c p) d -> p c d", p=128),
            )

        # ---- downsampled scores + exact per-row 128th largest ----
        # sds tile 0 serves query blocks 0-7 (groups 0-1); tile 1 serves
        # blocks 8-15 (groups 2-3) and is emitted only after group 0's
        # processing, so its 31 serial DVE max8/match_replace rounds overlap
        # group 0's PE/ACT-heavy PV phase instead of blocking the group-0
        # mask chains in the in-order DVE queue.
        sds_tiles = {}

        def sds_topk(t):
            sds = sdspool.tile([128, TDS + 1], F32, tag=f"sds{t}",
                               name=f"sds{t}")
            sds_tiles[t] = sds
            ps = ps2pool.tile([128, 512], F32, tag="ps2", name=f"sdsps{t}")
            nc.tensor.matmul(ps[:], qdst[:, 128 * t:128 * t + 128], kdst[:])
            nc.scalar.mul(sds[:, 0:TDS], ps[:], SCALE)
            scr = scrpool.tile([128, TDS], F32, tag="scr", name=f"scr{t}")
            nc.vector.tensor_copy(scr[:], sds[:, 0:TDS])
            maxsc = scrpool.tile([128, 8], F32, tag="maxsc", name=f"maxsc{t}")
            for rnd in range(KDS // 8):
                nc.vector.max(out=maxsc[:], in_=scr[:])
                if rnd < KDS // 8 - 1:
                    nc.vector.match_replace(
                        out=scr[:], in_to_replace=maxsc[:], in_values=scr[:],
                        imm_value=ZAP,
                    )
            nc.vector.tensor_copy(sds[:, TDS:TDS + 1], maxsc[:, 7:8])
            if dbg is not None:
                nc.sync.dma_start(out=dbg[f"SDS{t}"], in_=sds[:])

        sds_topk(0)
        full_prep()

        # ---- per-block processing: mask chain + scores + exp; PV per group.
        # Only the causally-allowed ds columns [0, 16*(kb+1)) are computed;
        # the tail of A is a constant -1e9.
        amins, negms, cees = {}, {}, {}
        ptiles, rsums = {}, {}

        def mask_chain(b):
            kb = KBS[r][b]
            nd = 16 * (kb + 1)
            tt, pp = divmod(b, 8)
            sds = sds_tiles[tt]
            ps = ps2pool.tile([128, 512], F32, tag="ps2")
            nc.tensor.matmul(ps[:, 0:nd], reps[pp][:], sds[:, 0:nd])
            ps1 = ps1pool.tile([128, 1], F32, tag="ps1")
            nc.tensor.matmul(ps1[:], reps[pp][:], sds[:, TDS:TDS + 1])
            thru = tinypool.tile([128, 1], F32, tag="thru")
            nc.scalar.mul(thru[:], ps1[:], usc[:, b:b + 1])
            # m = 1/(1 + exp(-10*arg)), arg = fl(fl(s*u) - fl(thr*u)) --
            # matches XLA's f32 sigmoid incl. the saturation-to-1.0 cliff
            # (the HW Sigmoid table is 1 ulp off there, which flips
            # (1-m)*(-1e9) between 0 and -59.6 and corrupts half the rows).
            arg = epool.tile([128, TDS], F32, tag="arg")
            nc.vector.tensor_scalar(arg[:, 0:nd], ps[:, 0:nd], usc[:, b:b + 1],
                                    thru[:], op0=ALU.mult, op1=ALU.subtract)
            z = epool.tile([128, TDS], F32, tag="z")
            nc.scalar.activation(z[:, 0:nd], arg[:, 0:nd], ACTF.Exp, scale=-10.0)
            nc.gpsimd.tensor_scalar(z[:, 0:nd], z[:, 0:nd], 1.0, None, op0=ALU.add)
            e = epool.tile([128, TDS], F32, tag="e")
            nc.vector.reciprocal(e[:, 0:nd], z[:, 0:nd])
            f = fpool.tile([128, TDS], F32, tag="f")
            # W = fl(1 - m) = fl(-m) + 1 (exact negate), F = fl(W * -1e9)
            nc.gpsimd.tensor_scalar(f[:, 0:nd], e[:, 0:nd], -1.0, 1.0,
                                    op0=ALU.mult, op1=ALU.add)
            nc.gpsimd.tensor_scalar(f[:, 0:nd], f[:, 0:nd], NEG, None,
                                    op0=ALU.mult)
            amin = aminpool.tile([128, TDS], F32, tag="amin")
            off = 496 - 16 * kb
            nc.vector.tensor_tensor(amin[:, 0:nd], f[:, 0:nd],
                                    cm3[:, off:off + nd], op=ALU.min)
            negm = smallpool.tile([128, 1], F32, tag="negm")
            nc.vector.tensor_reduce(negm[:], amin[:, 0:nd], axis=AX.X,
                                    op=ALU.max, negate=True)
            # per-row constant value of the suffix (fully-masked) columns:
            # C = exp(fl(-1e9 - M)); 0 for healthy rows, the uniform weight
            # for desperate rows.
            b2 = tinypool.tile([128, 1], F32, tag="b2")
            nc.vector.tensor_scalar(b2[:], negm[:], NEG, None, op0=ALU.add)
            cee = smallpool.tile([128, 1], F32, tag="cee")
            nc.scalar.activation(cee[:], b2[:], ACTF.Exp)
            amins[b] = amin
            negms[b] = negm
            cees[b] = cee
            # C^T for the rank-1 suffix inject
            if kb + 1 <= CH - 1:
                ctp = swppool.tile([1, 128], F32, tag="swp")
                nc.tensor.transpose(ctp[:], cee[:], ident[:])
                nc.scalar.copy(call[0:1, 128 * b:128 * b + 128], ctp[:])
            if dbg is not None and b == 0:
                nc.sync.dma_start(out=dbg["E0"], in_=e[:])
                nc.sync.dma_start(out=dbg["F0"], in_=f[:])
                nc.sync.dma_start(out=dbg["AM0"], in_=amin[:])

        def score_block(b):
            kb = KBS[r][b]
            ncol = 128 * (kb + 1)
            nsuf = T - ncol
            n512 = (ncol + 511) // 512
            p = ppool.tile([128, T], F32, tag="p")
            ptiles[b] = p
            for j in range(n512):
                lim = min(512, ncol - 512 * j)
                ps = ps2pool.tile([128, 512], F32, tag="ps2")
                nc.tensor.matmul(
                    ps[:, 0:lim],
                    qt[:, 128 * b:128 * b + 128],
                    kt[:, 512 * j:512 * j + lim],
                )
                # X = fl(fl(S*scale) + A), A = min(F, ds-causal) broadcast x8
                a_sl = amins[b][:, 64 * j:64 * j + lim // 8].unsqueeze(-1) \
                    .to_broadcast([128, lim // 8, STRIDE])
                x_v = p[:, 512 * j:512 * j + lim].rearrange(
                    "p (c s) -> p c s", s=STRIDE)
                nc.vector.scalar_tensor_tensor(
                    out=x_v, in0=ps[:, 0:lim].rearrange("p (c s) -> p c s",
                                                        s=STRIDE),
                    scalar=SCALE, in1=a_sl, op0=ALU.mult, op1=ALU.add)
                if 512 * j <= 128 * kb < 512 * j + lim:
                    dl = 128 * kb - 512 * j
                    tmp = tmppool.tile([128, 128], F32, tag="tmp")
                    nc.scalar.activation(tmp[:], ps[:, dl:dl + 128],
                                         ACTF.Identity, bias=negc[:], scale=SCALE)
                    nc.vector.copy_predicated(
                        p[:, 128 * kb:128 * kb + 128], c01inv[:], tmp[:])
            if dbg is not None and b == 0:
                nc.sync.dma_start(out=dbg["X0"], in_=p[:])
            ssum = tinypool.tile([128, 1], F32, tag="ssum")
            nc.scalar.activation(p[:, 0:ncol], p[:, 0:ncol], ACTF.Exp,
                                 bias=negms[b][:], scale=1.0, accum_out=ssum[:])
            if dbg is not None and b == 0:
                nc.sync.dma_start(out=dbg["P0"], in_=p[:])
                nc.sync.dma_start(out=dbg["NM0"], in_=negms[b][:])
                nc.sync.dma_start(out=dbg["SS0"], in_=ssum[:])
            # denominator = mainsum + C * n_suffix
            den = tinypool.tile([128, 1], F32, tag="den")
            nc.vector.scalar_tensor_tensor(
                out=den[:], in0=cees[b][:], scalar=float(nsuf), in1=ssum[:],
                op0=ALU.mult, op1=ALU.add)
            rsum = smallpool.tile([128, 1], F32, tag="rsum")
            nc.vector.reciprocal(rsum[:], den[:])
            rsums[b] = rsum

        osb = inpool.tile([128, NB * 128], F32, tag="osb")
        onesr = cpool.tile([128, 1], F32R, tag="onesr")
        onesf = cpool.tile([128, 1], F32, tag="onesf")
        nc.gpsimd.memset(onesf[:], 1.0)
        nc.vector.tensor_copy(onesr[:], onesf[:])
        swall = swsbpool.tile([1, NB * 128], F32R, tag="swall")
        call = swsbpool.tile([1, NB * 128], F32R, tag="call")

        # ---- suffix V column-sums SW(cb) = sum_{c >= cb} V[c-chunk],
        # stored as [1, 128]-slices of a partition-0 tile ----
        swp = swppool.tile([1, 128], F32, tag="swp")
        emitted = 0
        prev = CH
        for b in range(NB - 1, -1, -1):
            cb = KBS[r][b] + 1
            for c in range(cb, prev):
                emitted += 1
                # stop before each snapshot read (sim requirement); the psum
                # keeps accumulating across groups via start=False.
                nc.tensor.matmul(swp[:], onesr[:], vsb[:, 128 * c:128 * c + 128],
                                 start=(emitted == 1), stop=(c == cb),
                                 skip_group_check=(emitted != 1))
            prev = cb
            if cb <= CH - 1:
                nc.scalar.copy(swall[0:1, 128 * b:128 * b + 128], swp[:])

        # ---- interleaved main loop: per group, run the 4 blocks' mask
        # chain + scores + exp, then the group's PV.  Chunk c feeds only
        # blocks with kb >= c; the suffix columns contribute the rank-1
        # term SW(kb+1) (x) C, injected directly into the PSUM accumulator.
        def pv_emit(g, jlo, jhi, outt):
            """PV accumulation for blocks 4g+jlo .. 4g+jhi into outt columns
            [128*jlo, 128*(jhi+1)); suffix rank-1 injects included."""
            kbs = [KBS[r][4 * g + j] for j in range(jlo, jhi + 1)]
            cmax = kbs[-1]
            for c in range(cmax + 1):
                jmin = jlo + min(i for i, kb in enumerate(kbs) if kb >= c)
                hi = 128 * (jhi + 1)
                ptp = ptpool.tile([128, 512], F32, tag="ptp")
                for j in range(jmin, jhi + 1):
                    nc.tensor.transpose(
                        ptp[:, 128 * j:128 * j + 128],
                        ptiles[4 * g + j][:, 128 * c:128 * c + 128],
                        ident[:],
                    )
                pts = ptsbpool.tile([128, 512], F32R, tag="pts")
                mod = 2 if jlo != 0 else 8
                if c % mod == 0:
                    nc.vector.tensor_copy(pts[:, 128 * jmin:hi],
                                          ptp[:, 128 * jmin:hi])
                else:
                    nc.scalar.copy(pts[:, 128 * jmin:hi],
                                   ptp[:, 128 * jmin:hi])
                nc.tensor.matmul(
                    outt[:, 128 * jmin:hi],
                    vsb[:, 128 * c:128 * c + 128],
                    pts[:, 128 * jmin:hi],
                    start=(c == 0), stop=(c == cmax),
                    skip_group_check=(jlo != 0),
                )
            for j in range(jlo, jhi + 1):
                b = 4 * g + j
                if KBS[r][b] + 1 <= CH - 1:
                    nc.tensor.matmul(
                        outt[:, 128 * j:128 * j + 128],
                        swall[0:1, 128 * b:128 * b + 128],
                        call[0:1, 128 * b:128 * b + 128],
                        start=False, stop=True, skip_group_check=True,
                    )

        def pv_epilogue(g, outt):
            outt_sb = outtsbpool.tile([128, 512], F32, tag="outtsb")
            nc.any.tensor_copy(outt_sb[:], outt[:])
            ops = ptpool.tile([128, 512], F32, tag="ptp")
            for j in range(4):
                nc.tensor.transpose(
                    ops[:, 128 * j:128 * j + 128],
                    outt_sb[:, 128 * j:128 * j + 128], ident[:])
            for j in range(4):
                b = 4 * g + j
                nc.scalar.mul(osb[:, 128 * b:128 * b + 128],
                              ops[:, 128 * j:128 * j + 128], rsums[b][:])
                row0 = 128 * b
                nc.sync.dma_start(out=Od[row0:row0 + 128, :],
                                  in_=osb[:, 128 * b:128 * b + 128])

        for gi, g in enumerate(GORDERS[r]):
            last = gi >= len(GORDERS[r]) - 2
            outt = outtpool.tile([128, 512], F32, tag="outt")
            if last:
                # split the final group's PV so its first half overlaps the
                # last two blocks' scores/exp (shortens the serial tail)
                for j in range(2):
                    mask_chain(4 * g + j)
                    score_block(4 * g + j)
                pv_emit(g, 0, 1, outt)
                for j in range(2, 4):
                    mask_chain(4 * g + j)
                    score_block(4 * g + j)
                pv_emit(g, 2, 3, outt)
            else:
                for j in range(4):
                    mask_chain(4 * g + j)
                    score_block(4 * g + j)
                pv_emit(g, 0, 3, outt)
            pv_epilogue(g, outt)
            if gi == 0:
                sds_topk(1)


_PROGRAMS = {}


def build_program(r: int, debug=False):
    key = (r, debug)
    if key in _PROGRAMS:
        return _PROGRAMS[key]
    nc = bacc.Bacc("TRN2", target_bir_lowering=False, debug=False)
    Qd = nc.dram_tensor("Q", [NLQ, HD], F32, kind="ExternalInput").ap()
    Kd = nc.dram_tensor("K", [T, HD], F32, kind="ExternalInput").ap()
    Vd = nc.dram_tensor("V", [T, HD], F32R, kind="ExternalInput").ap()
    Ud = nc.dram_tensor("UBT", [128, NB], F32, kind="ExternalInput").ap()
    Od = nc.dram_tensor("OUT", [NLQ, HD], F32, kind="ExternalOutput").ap()
    dbg = None
    if debug:
        dbg = {}
        for nm, shp in (("SDS0", [128, TDS + 1]), ("SDS1", [128, TDS + 1]),
                        ("E0", [128, TDS]),
                        ("F0", [128, TDS]), ("AM0", [128, TDS]),
                        ("X0", [128, T]), ("P0", [128, T]),
                        ("NM0", [128, 1]), ("SS0", [128, 1])):
            dbg[nm] = nc.dram_tensor(nm, shp, F32, kind="ExternalOutput").ap()
    with tile.TileContext(nc) as tc:
        _kernel_body(tc, r, Qd, Kd, Vd, Ud, Od, dbg)
    nc.compile()
    _PROGRAMS[key] = nc
    return nc


def shard_inputs(Q, K, V, U):
    """Return per-core input dicts: core = 4*r + h (devices 0-3 parity 0)."""
    maps = []
    Qr = Q[0].reshape(NH, 2 * NB, QPB, HD)
    Ur = U[0].reshape(2 * NB, QPB)
    for r in range(2):
        for h in range(NH):
            qsh = np.ascontiguousarray(Qr[h, KBS[r]].reshape(NLQ, HD))
            ubt = np.ascontiguousarray(Ur[KBS[r]].T)        # [128, NB]
            maps.append({
                "Q": qsh,
                "K": np.ascontiguousarray(K[0, h]),
                "V": np.ascontiguousarray(V[0, h]),
                "UBT": ubt,
            })
    return maps


def unshard_output(outs):
    """outs: list of 8 dicts with 'OUT' [2048, 128] in core order above."""
    O = np.empty((B, NH, T, HD), np.float32)
    Ov = O[0].reshape(NH, 2 * NB, QPB, HD)
    i = 0
    for r in range(2):
        for h in range(NH):
            Ov[h, KBS[r]] = outs[i]["OUT"].reshape(NB, QPB, HD)
            i += 1
    return O


def _run_concurrent(in_maps):
    """Dispatch parity-0 program on devices 0-3 and parity-1 on devices 4-7,
    concurrently (adapted from concourse.bass2jax.run_bass_via_pjrt)."""
    import jax
    from jax.sharding import Mesh, PartitionSpec
    from jax.experimental.shard_map import shard_map
    from concourse import bass2jax

    bass2jax.install_neuronx_cc_hook()
    devices = jax.devices()
    assert len(devices) >= 8, f"need 8 neuron cores, got {len(devices)}"

    pending = []
    for r in range(2):
        nc = build_program(r)
        maps = in_maps[4 * r:4 * r + 4]
        pname = nc.partition_id_tensor.name if nc.partition_id_tensor else None
        in_names, out_names, out_avals, zero_outs = [], [], [], []
        for alloc in nc.m.functions[0].allocations:
            if not isinstance(alloc, mybir.MemoryLocationSet):
                continue
            name = alloc.memorylocations[0].name
            if alloc.kind == "ExternalInput":
                if name != pname:
                    in_names.append(name)
            elif alloc.kind == "ExternalOutput":
                out_names.append(name)
                shape = tuple(alloc.tensor_shape)
                dtype = mybir.dt.np(alloc.dtype)
                out_avals.append(jax.core.ShapedArray(shape, dtype))
                zero_outs.append(np.zeros(shape, dtype))
        n_params = len(in_names)
        n_outs = len(out_avals)
        all_names = in_names + out_names
        if pname is not None:
            all_names = all_names + [pname]
        donate = tuple(range(n_params, n_params + n_outs))

        def _body(*args, _nc=nc, _avals=tuple(out_avals),
                  _names=tuple(all_names), _onames=tuple(out_names),
                  _pname=pname):
            operands = list(args)
            if _pname is not None:
                operands.append(bass2jax.partition_id_tensor())
            outs = bass2jax._bass_exec_p.bind(
                *operands,
                out_avals=_avals,
                in_names=_names,
                out_names=_onames,
                lowering_input_output_aliases=(),
                sim_require_finite=True,
                sim_require_nnan=True,
                nc=_nc,
            )
            return tuple(outs)

        mesh = Mesh(np.asarray(devices[4 * r:4 * r + 4]), ("core",))
        in_specs = (PartitionSpec("core"),) * (n_params + n_outs)
        out_specs = (PartitionSpec("core"),) * n_outs
        fn = jax.jit(
            shard_map(_body, mesh=mesh, in_specs=in_specs,
                      out_specs=out_specs, check_rep=False),
            donate_argnums=donate, keep_unused=True,
        )
        per_core = [[np.asarray(m[nm]) for nm in in_names] for m in maps]
        concat_in = [
            np.concatenate([per_core[c][i] for c in range(4)], axis=0)
            for i in range(n_params)
        ]
        concat_zero = [
            np.concatenate([z] * 4, axis=0) for z in zero_outs
        ]
        out_arrs = fn(*concat_in, *concat_zero)   # async dispatch
        pending.append((out_arrs, out_names))

    results = []
    for r, (out_arrs, out_names) in enumerate(pending):
        outs = [np.asarray(a) for a in out_arrs]   # force
        for c in range(4):
            d = {}
            for i, nm in enumerate(out_names):
                n0 = outs[i].shape[0] // 4
                d[nm] = outs[i][c * n0:(c + 1) * n0]
            results.append(d)
    return results


def kernel(**inputs):
    Q = np.asarray(inputs["Q"], np.float32)
    K = np.asarray(inputs["K"], np.float32)
    V = np.asarray(inputs["V"], np.float32)
    U = np.asarray(inputs["U"], np.float32)
    in_maps = shard_inputs(Q, K, V, U)
    results = _run_concurrent(in_maps)
    return unshard_output(results)



# revision 4
# speedup vs baseline: 1.4254x; 1.4254x over previous
"""DSALite sparse-attention Trainium2 kernel, transposed-flow redesign.

Problem: B=1, nH=4, T=4096, hd=128 attention where the mask is derived from
8x-downsampled scores: per full row, threshold = 128th largest of the 512
downsampled (and u-scaled) scores, mask = sigmoid((s - thr) * 10 * u) * causal,
scores += (1-mask) * (-1e9), softmax, @V.

Sharding: 8 cores = 4 heads x 2 row-parities (identical to the baseline
kernel).  Core (h, r) handles head h and the 16 query blocks KBS[r].

v2 redesign (vs the q-major baseline): all per-cell work runs in the
TRANSPOSED domain X^T[k, q] so the mask add, the row-max subtraction and the
softmax denominator come out of PE matmuls instead of DVE elementwise ops:

  per 128-k chunk c, per 4-block group (512 q):
    X^T  = K_c Q^T           (fp32r matmul, scale folded into Q^T)
         + Rep(c) @ amin2T   (replicates 16 ds-mask rows onto 128 k rows;
                              amin2 = min(F, ds-causal) + negm pre-folded)
    P^T  = exp(X^T)          (one ACT op PSUM->SBUF(f32r), no bias needed)
    diag chunk: zero k>q half via one gpsimd affine_select (reference
                contributes exactly 0 there: exp(S - 1e9 - M) underflows)
    den += ones^T @ P^T      (PE rank-1 into a [1,512] accumulator)
    O^T += V_c^T @ P^T       (fp32r accumulation, 512 wide)

This deletes the baseline's three big serial DVE/ACT burdens: the X = S+A
elementwise add (51us DVE), the P chunk transposes (34k PE cycles), and the
PSUM->SBUF P^T copies (~20us DVE + ACT).  The mask chain itself (exact f32
sigmoid bucket semantics, top-128 threshold via DVE max8/match_replace) is
carried over op-for-op from the baseline; amin2T is produced by 40 small PE
transposes of the per-block amin2 = amin + negm tiles.

Numerical notes (same bucket-exactness strategy as the baseline):
  - ds scores/threshold/sigmoid/F chain identical to baseline (exact f32).
  - X^T accumulates S~fp32r + amin2 + negm in f32 PSUM adds; only the smooth
    softmax path sees the fp32r rounding, mask buckets are computed exactly.
  - denominator comes from ones@P^T in fp32r (P in [0,1], err ~2^-21 rel).
  - suffix (fully-masked) columns contribute C = exp(-1e9 - M) per row via
    rank-1 SW x C^T injects and a C * n_suffix denominator fixup (baseline
    mechanism, unchanged).
"""

import numpy as np

import concourse.bass as bass
import concourse.bacc as bacc
import concourse.mybir as mybir
import concourse.tile as tile
from concourse.masks import make_identity

F32 = mybir.dt.float32
F32R = mybir.dt.float32r
BF16 = mybir.dt.bfloat16
ALU = mybir.AluOpType
ACTF = mybir.ActivationFunctionType
AX = mybir.AxisListType

B, NH, T, HD = 1, 4, 4096, 128
STRIDE = 8
TDS = T // STRIDE          # 512 downsampled positions
KDS = 128                  # exact 128th largest per ds row
NEG = -1e9
SCALE = HD ** -0.5
ZAP = -1e30

NB = 16                    # 128-row query blocks per core
QPB = 128
NLQ = NB * QPB             # 2048 local query rows
NG = 4                     # groups of 4 blocks (512 q)
CH = T // 128              # 32 key/value chunks
CMW = 1008                 # sliding ds-causal const width
GORDERS = {0: [0, 1, 2, 3], 1: [0, 1, 2, 3]}
# Same per-program block SETS as the baseline (near-equal causal work), but
# ordered so sds-tile 0 (list positions 0-7 = groups A,B) carries the bulk of
# the PE work: group B's big blocks keep PE busy for the ~19us that tile 1's
# serial top-k chain occupies the DVE.  Groups ascend within themselves
# (the jmin narrowing logic requires it); block 0 of the list must be the
# program's earliest block (desperate-row special handling).
KBS = [
    [0, 2, 4, 6, 24, 27, 29, 31, 8, 10, 12, 14, 18, 20, 22, 23],
    [1, 3, 5, 7, 25, 26, 28, 30, 9, 11, 13, 15, 16, 17, 19, 21],
]


def _consts(nc, pool):
    ident = pool.tile([128, 128], F32, tag="ident")
    make_identity(nc, ident[:])
    identr = pool.tile([128, 128], F32R, tag="identr")
    nc.vector.tensor_copy(identr[:], ident[:])

    # cm3[i, jj] = 0.0 if jj <= 496 + i//8 else -1e9 (sliding ds-causal mask)
    cm3 = pool.tile([128, CMW], F32, tag="cm3")
    nc.gpsimd.memset(cm3[:], 0.0)
    nc.gpsimd.affine_select(
        out=cm3[:], in_=cm3[:], pattern=[[-8, CMW]], base=3968,
        channel_multiplier=1, compare_op=ALU.is_ge, fill=NEG,
    )

    # rep[bp][k, i] = 1.0 iff k == 16*bp + i//8 (f32 for the mask chain,
    # f32r copies for the X^T mask-replication matmuls)
    reps, repsr = [], []
    for bp in range(8):
        rep = pool.tile([128, 128], F32, tag=f"rep{bp}")
        nc.gpsimd.memset(rep[:], 1.0)
        nc.gpsimd.affine_select(
            out=rep[:], in_=rep[:], pattern=[[1, 128]], base=128 * bp,
            channel_multiplier=-8, compare_op=ALU.is_ge, fill=0.0)
        nc.gpsimd.affine_select(
            out=rep[:], in_=rep[:], pattern=[[-1, 128]], base=7 - 128 * bp,
            channel_multiplier=8, compare_op=ALU.is_ge, fill=0.0)
        reps.append(rep)
        repr_ = pool.tile([128, 128], F32R, tag=f"repr{bp}")
        nc.vector.tensor_copy(repr_[:], rep[:])
        repsr.append(repr_)

    onesf = pool.tile([128, 1], F32, tag="onesf")
    nc.gpsimd.memset(onesf[:], 1.0)
    onesr = pool.tile([128, 1], F32R, tag="onesr")
    nc.vector.tensor_copy(onesr[:], onesf[:])
    onesb = pool.tile([128, 1], BF16, tag="onesb")
    nc.gpsimd.memset(onesb[:], 1.0)
    onesrow = pool.tile([1, 128], F32, tag="onesrow")
    nc.gpsimd.memset(onesrow[:], 1.0)

    # c01T[i, j] = 1 where i > j (strictly below diagonal in [k, q] layout =
    # above-diagonal in q-major): the region of the special block's diagonal
    # chunk overwritten with the per-row masked constant C.
    c01t = pool.tile([128, 128], mybir.dt.int8, tag="c01t")
    nc.gpsimd.memset(c01t[:], 1)
    nc.gpsimd.affine_select(
        out=c01t[:], in_=c01t[:], pattern=[[-1, 128]], base=-1,
        channel_multiplier=1, compare_op=ALU.is_ge, fill=0,
    )

    return ident, identr, cm3, reps, repsr, onesr, onesb, onesrow, c01t


def _kernel_body(tc, r, Qd, Kd, QDSd, KDSd, Vd, Ud, Od, dbg=None):
    nc = tc.nc
    from contextlib import ExitStack
    with ExitStack() as ctx:
        cpool = ctx.enter_context(tc.tile_pool(name="consts", bufs=1))
        inpool = ctx.enter_context(tc.tile_pool(name="inputs", bufs=1))
        # PSUM budget (8 banks): xps 2 + outt 1 + rsps 1 + maskps 2 + ptps 1
        # + scratch 1
        xpool = ctx.enter_context(tc.tile_pool(name="xps", bufs=2, space="PSUM"))
        outtpool = ctx.enter_context(tc.tile_pool(name="outtps", bufs=1, space="PSUM"))
        rspool = ctx.enter_context(tc.tile_pool(name="rsps", bufs=1, space="PSUM"))
        mpspool = ctx.enter_context(tc.tile_pool(name="maskps", bufs=2, space="PSUM"))
        ptpool = ctx.enter_context(tc.tile_pool(name="ptps", bufs=1, space="PSUM"))
        scrpspool = ctx.enter_context(tc.tile_pool(name="scrps", bufs=1, space="PSUM"))
        sdspool = ctx.enter_context(tc.tile_pool(name="sds", bufs=1))
        scrpool = ctx.enter_context(tc.tile_pool(name="scr", bufs=1))
        epool = ctx.enter_context(tc.tile_pool(name="e", bufs=3))
        fpool = ctx.enter_context(tc.tile_pool(name="f", bufs=5))
        aminpool = ctx.enter_context(tc.tile_pool(name="amin", bufs=3))
        am2pool = ctx.enter_context(tc.tile_pool(name="am2", bufs=8))
        amtpool = ctx.enter_context(tc.tile_pool(name="amt", bufs=7))
        smallpool = ctx.enter_context(tc.tile_pool(name="small", bufs=NB))
        tinypool = ctx.enter_context(tc.tile_pool(name="tiny", bufs=4))
        ptspool = ctx.enter_context(tc.tile_pool(name="pts", bufs=4))
        outtsbpool = ctx.enter_context(tc.tile_pool(name="outtsb", bufs=2))
        swsbpool = ctx.enter_context(tc.tile_pool(name="swsb", bufs=1))

        (ident, identr, cm3, reps, repsr, onesr, onesb, onesrow,
         c01t) = _consts(nc, cpool)

        # one shared PSUM scratch bank; disjoint regions, subtile-dep tracked
        scrps = scrpspool.tile([128, 512], F32, tag="scrps")
        swp = scrps[0:1, 0:128]        # suffix colsum accumulator
        ps1b_r = scrps[0:128, 128:129]  # threshold replicate matmul out
        ctp_r = scrps[0:1, 192:320]     # cee^T transpose out
        rst_r = scrps[0:128, 352:353]   # rowsum^T transpose out
        ngt_r = scrps[0:1, 384:512]     # negm^T transpose out (block 0)

        # ---- loads: ds subsets first (they gate the serial top-k chain) ----
        kt = inpool.tile([128, T], F32R, tag="kt")     # K^T [d, t]
        qts = inpool.tile([128, NLQ], F32R, tag="qts")  # Q^T * scale [d, q]
        kdst = inpool.tile([128, TDS], F32, tag="kdst")
        qdst = inpool.tile([128, TDS // 2], F32, tag="qdst")
        prep = ctx.enter_context(tc.tile_pool(name="prep", bufs=2))
        for dsrc, dsdst, nds in ((KDSd, kdst, TDS), (QDSd, qdst, TDS // 2)):
            natd = prep.tile([128, nds], F32, tag="natd", name="natd")
            nc.sync.dma_start(
                out=natd[:].rearrange("p (c d) -> p c d", d=128),
                in_=dsrc.rearrange("(c p s) d -> p c s d", p=128,
                                   s=STRIDE)[:, :, 0, :],
            )
            n3 = natd[:].rearrange("p (c d) -> p c d", d=128)
            ptd = ptpool.tile([128, 512], F32, tag="ptps", name="ppsd")
            for j in range(nds // 128):
                nc.tensor.transpose(
                    ptd[:, 128 * j:128 * j + 128], n3[:, j, :], ident[:])
            # ACT copy: keeps the DVE queue free so topk0 starts immediately
            nc.scalar.copy(dsdst[:], ptd[:, 0:nds])

        ub = inpool.tile([128, NB], F32, tag="ub")
        nc.sync.dma_start(out=ub[:], in_=Ud[:])
        usc = inpool.tile([128, NB], F32, tag="usc")
        nc.vector.tensor_scalar(usc[:], ub[:], 0.0, 1.0, op0=ALU.max,
                                op1=ALU.min)
        nc.vector.tensor_scalar(usc[:], usc[:], 1.0, None, op0=ALU.add)
        vsb = inpool.tile([128, T], F32R, tag="vsb")   # [t, d] natural

        def v_load():
            # V early: the suffix colsums are the main PE work available
            # under the serial topk0 DVE chain
            nc.sync.dma_start(
                out=vsb[:].rearrange("p (c d) -> p c d", d=128),
                in_=Vd.rearrange("(c p) d -> p c d", p=128),
            )

        def full_prep():
            # K^T: 8 pieces of 512 cols; f32r transposes (1.5 cyc/row)
            s3 = Kd.rearrange("(c p) d -> p c d", p=128)
            for c4 in range(8):
                nat = prep.tile([128, 512], F32R, tag="nat")
                nc.sync.dma_start(
                    out=nat[:].rearrange("p (c d) -> p c d", d=128),
                    in_=s3[:, 4 * c4:4 * c4 + 4, :],
                )
                n3 = nat[:].rearrange("p (c d) -> p c d", d=128)
                pt = ptpool.tile([128, 512], F32R, tag="ptps")
                for j in range(4):
                    nc.tensor.transpose(
                        pt[:, 128 * j:128 * j + 128], n3[:, j, :], identr[:])
                nc.any.tensor_copy(kt[:, 512 * c4:512 * c4 + 512], pt[:])
            # Q^T with the score scale folded into the PSUM->SBUF copy
            s3 = Qd.rearrange("(c p) d -> p c d", p=128)
            for c4 in range(4):
                nat = prep.tile([128, 512], F32R, tag="nat")
                nc.sync.dma_start(
                    out=nat[:].rearrange("p (c d) -> p c d", d=128),
                    in_=s3[:, 4 * c4:4 * c4 + 4, :],
                )
                n3 = nat[:].rearrange("p (c d) -> p c d", d=128)
                pt = ptpool.tile([128, 512], F32R, tag="ptps")
                for j in range(4):
                    nc.tensor.transpose(
                        pt[:, 128 * j:128 * j + 128], n3[:, j, :], identr[:])
                nc.scalar.mul(qts[:, 512 * c4:512 * c4 + 512], pt[:], SCALE)

        # ---- downsampled scores + exact per-row 128th largest ----
        # Split so sds matmuls (PE) can be emitted early while the serial
        # DVE top-k rounds are placed where the DVE queue wants them.
        sds_tiles = {}

        def sds_mm(t):
            sds = sdspool.tile([128, TDS + 1], F32, tag=f"sds{t}",
                               name=f"sds{t}")
            sds_tiles[t] = sds
            ps = mpspool.tile([128, 512], F32, tag="maskps", name=f"sdsps{t}")
            nc.tensor.matmul(ps[:], qdst[:, 128 * t:128 * t + 128], kdst[:])
            nc.scalar.mul(sds[:, 0:TDS], ps[:], SCALE)

        def topk_rounds(t):
            sds = sds_tiles[t]
            scr = scrpool.tile([128, TDS], F32, tag="scr", name=f"scr{t}")
            nc.vector.tensor_copy(scr[:], sds[:, 0:TDS])
            maxsc = scrpool.tile([128, 8], F32, tag="maxsc", name=f"maxsc{t}")
            for rnd in range(KDS // 8):
                nc.vector.max(out=maxsc[:], in_=scr[:])
                if rnd < KDS // 8 - 1:
                    nc.vector.match_replace(
                        out=scr[:], in_to_replace=maxsc[:], in_values=scr[:],
                        imm_value=ZAP,
                    )
            nc.vector.tensor_copy(sds[:, TDS:TDS + 1], maxsc[:, 7:8])
            if dbg is not None:
                nc.sync.dma_start(out=dbg[f"SDS{t}"], in_=sds[:])

        sds_mm(0)
        topk_rounds(0)
        v_load()
        full_prep()

        # ---- suffix V column-sums SW(cb), [1, 128] slices in swall ----
        # bf16 V copy makes the narrow [1,128]-out colsum matmuls 1 cyc/row
        # (fp32r pays a 4x penalty below 256-wide); SW only feeds desperate
        # rows via C = exp(-1e9 - M), so 0.4% bf16 rounding is irrelevant.
        osb = inpool.tile([128, NB * 128], F32, tag="osb")
        vsbb = inpool.tile([128, T], BF16, tag="vsbb")
        nc.gpsimd.tensor_copy(vsbb[:], vsb[:])
        swall = swsbpool.tile([1, NB * 128], BF16, tag="swall")
        call = swsbpool.tile([1, NB * 128], BF16, tag="call")
        negmrow4 = swsbpool.tile([1, 512], F32, tag="negmrow4")
        ceerow0 = swsbpool.tile([1, 128], F32, tag="ceerow0")
        amtf = inpool.tile([128, 512], F32, tag="amtf")   # gA amin^T (f32)
        # cbm[k, q] = C(q) for k > q else 0: the diag chunk's above-diagonal
        # dust of the desperate-capable block, injected via two rank-128
        # matmuls into rsps and outt (pts itself stays affine-zeroed there)
        cbm = inpool.tile([128, 128], F32R, tag="cbm")
        emitted = 0
        prev = CH
        for b in sorted(range(NB), key=lambda b_: KBS[r][b_], reverse=True):
            cb = KBS[r][b] + 1
            for c in range(cb, prev):
                emitted += 1
                nc.tensor.matmul(swp, onesb[:], vsbb[:, 128 * c:128 * c + 128],
                                 start=(emitted == 1), stop=(c == cb),
                                 skip_group_check=(emitted != 1))
            prev = cb
            if cb <= CH - 1:
                nc.scalar.copy(swall[0:1, 128 * b:128 * b + 128], swp)

        # ---- mask chain (q-major, bucket-exact; -> amin2 f32r + cee) ----
        # Emitted as per-block STEP thunks so 8 blocks' chains interleave
        # (pipelining the 12-step cross-engine latency chain) and so steps can
        # be pumped into the chunk pipelines of earlier groups.
        negms, cees, am2s = {}, {}, {}

        def chain_steps(b):
            kb = KBS[r][b]
            nd = 16 * (kb + 1)
            tt, pp = divmod(b, 8)
            st = {}

            def s0():
                sds = sds_tiles[tt]
                st["ps"] = mpspool.tile([128, 512], F32, tag="maskps", name="chps")
                nc.tensor.matmul(st["ps"][:, 0:nd], reps[pp][:], sds[:, 0:nd])
                nc.tensor.matmul(ps1b_r, reps[pp][:], sds[:, TDS:TDS + 1])
                st["thru"] = tinypool.tile([128, 1], F32, tag="thru", name="thru")
                nc.scalar.mul(st["thru"][:], ps1b_r, usc[:, b:b + 1])

            def s1():
                # m = 1/(1 + exp(-10*arg)); bucket-exact vs XLA f32 sigmoid
                st["arg"] = epool.tile([128, TDS], F32, tag="arg", name="arg")
                nc.vector.tensor_scalar(st["arg"][:, 0:nd], st["ps"][:, 0:nd],
                                        usc[:, b:b + 1], st["thru"][:],
                                        op0=ALU.mult, op1=ALU.subtract)

            def s2():
                st["z"] = epool.tile([128, TDS], F32, tag="z", name="z")
                nc.scalar.activation(st["z"][:, 0:nd], st["arg"][:, 0:nd],
                                     ACTF.Exp, scale=-10.0)

            def s3():
                nc.gpsimd.tensor_scalar(st["z"][:, 0:nd], st["z"][:, 0:nd],
                                        1.0, None, op0=ALU.add)

            def s4():
                st["e"] = epool.tile([128, TDS], F32, tag="e", name="e")
                nc.vector.reciprocal(st["e"][:, 0:nd], st["z"][:, 0:nd])

            def s5():
                st["f"] = fpool.tile([128, TDS], F32, tag="f", name="f")
                nc.gpsimd.tensor_scalar(st["f"][:, 0:nd], st["e"][:, 0:nd],
                                        -1.0, 1.0, op0=ALU.mult, op1=ALU.add)

            def s6():
                nc.gpsimd.tensor_scalar(st["f"][:, 0:nd], st["f"][:, 0:nd],
                                        NEG, None, op0=ALU.mult)

            def s7():
                # amin = min(F, ds-causal): F <= 0 always and cm3 is 0 on all
                # fully-allowed columns, so the min only matters on the last
                # 16 ds cols (the diagonal ds band, cm3[:, 496:512]) -- apply
                # it in place on f, 16 cols instead of nd.
                nc.vector.tensor_tensor(st["f"][:, nd - 16:nd],
                                        st["f"][:, nd - 16:nd],
                                        cm3[:, 496:512], op=ALU.min)
                st["amin"] = st["f"]

            def s8():
                negm = smallpool.tile([128, 1], F32, tag="negm")
                nc.vector.tensor_reduce(negm[:], st["amin"][:, 0:nd], axis=AX.X,
                                        op=ALU.max, negate=True)
                negms[b] = negm

            def s9():
                if b < 4:
                    # group A blocks can hold desperate or SEMI-desperate
                    # rows (|max F| large): the reference's fl(S + F)
                    # quantizes S at ulp(F), so F must stay UNFOLDED in
                    # exact f32 and negm must be added only after the S+F
                    # rounding (fl(fl(S+F) + negm), matching the ref)
                    amu = am2pool.tile([128, TDS], F32, tag="am2", name="amu")
                    nc.vector.tensor_copy(amu[:, 0:nd], st["amin"][:, 0:nd])
                    am2s[b] = amu
                    return
                # amin2 = amin + negm (X^T = S + amin2 needs no later bias);
                # safe here: every masked column of these blocks has C == 0
                am2 = am2pool.tile([128, TDS], F32R, tag="am2", name="am2")
                nc.gpsimd.tensor_scalar(am2[:, 0:nd], st["amin"][:, 0:nd],
                                        negms[b][:], None, op0=ALU.add)
                am2s[b] = am2

            def s10():
                b2 = tinypool.tile([128, 1], F32, tag="b2")
                nc.vector.tensor_scalar(b2[:], negms[b][:], NEG, None,
                                        op0=ALU.add)
                cee = smallpool.tile([128, 1], F32, tag="cee")
                nc.scalar.activation(cee[:], b2[:], ACTF.Exp)
                cees[b] = cee

            def s11():
                kb_ = KBS[r][b]
                if kb_ + 1 <= CH - 1:
                    nc.tensor.transpose(ctp_r, cees[b][:], ident[:])
                    nc.scalar.copy(call[0:1, 128 * b:128 * b + 128], ctp_r)
                if b < 4:
                    # f32 rows for the exact rank-1 negm inject (+ C bcast)
                    if b == 0:
                        nc.scalar.copy(ceerow0[:], ctp_r)
                    nc.tensor.transpose(ngt_r, negms[b][:], ident[:])
                    nc.scalar.copy(negmrow4[0:1, 128 * b:128 * b + 128],
                                   ngt_r)
                if dbg is not None and b == 0:
                    nc.sync.dma_start(out=dbg["E0"], in_=st["e"][:])
                    nc.sync.dma_start(out=dbg["AM0"], in_=st["amin"][:])

            return [s0, s1, s2, s3, s4, s5, s6, s7, s8, s9, s10, s11]

        def interleave_chains(blocks):
            """Wavefront-interleaved chain thunks: chain i runs one step
            behind chain i-1, so pool rotation stays alloc-after-consumer
            and every wave packs different engines."""
            chains = [chain_steps(b) for b in blocks]
            out = []
            for w in range(len(chains) + 11):
                for i, ch in enumerate(chains):
                    s = w - i
                    if 0 <= s < 12:
                        out.append(ch[s])
            return out

        def amt_steps(g):
            """amin2T piece-build thunks for group g; returns (thunks, amts)."""
            kbs = [KBS[r][4 * g + j] for j in range(4)]
            cmax = kbs[3]
            npieces = (cmax + 1 + 7) // 8
            amts = []
            thunks = []
            if g == 0:
                # group A: one exact-f32 amin^T tile for all four blocks
                # (no folded pieces) + block 0's masked-C broadcast tile
                def build_exact(j):
                    def run():
                        b = j
                        nd = 16 * (kbs[j] + 1)
                        pt = ptpool.tile([128, 512], F32, tag="ptps",
                                         name="ptspec")
                        nc.tensor.transpose(pt[0:nd, 128 * j:128 * j + 128],
                                            am2s[b][:, 0:nd], ident[:])
                        nc.any.tensor_copy(
                            amtf[0:nd, 128 * j:128 * j + 128],
                            pt[0:nd, 128 * j:128 * j + 128])
                        if j == 0:
                            nc.tensor.matmul(pt[:, 128:256], onesrow[:],
                                             ceerow0[:])
                            nc.vector.tensor_copy(cbm[:], pt[:, 128:256])
                            nc.gpsimd.affine_select(
                                out=cbm[:], in_=cbm[:], pattern=[[-1, 128]],
                                base=-1, channel_multiplier=1,
                                compare_op=ALU.is_ge, fill=0.0)
                    return run
                for j in range(4):
                    thunks.append(build_exact(j))
                return thunks, amts
            for p in range(npieces):
                amt = amtpool.tile([128, 512], F32R, tag="amt",
                                   name=f"amt{g}_{p}")
                amts.append(amt)

                def build(p=p, amt=amt):
                    pt = ptpool.tile([128, 512], F32R, tag="ptps")
                    for j in range(4):
                        b = 4 * g + j
                        nd = 16 * (kbs[j] + 1)
                        w = min(128, nd - 128 * p)
                        if w <= 0:
                            continue
                        nc.tensor.transpose(
                            pt[0:w, 128 * j:128 * j + 128],
                            am2s[b][:, 128 * p:128 * p + w], identr[:])
                        nc.any.tensor_copy(amt[0:w, 128 * j:128 * j + 128],
                                           pt[0:w, 128 * j:128 * j + 128])
                thunks.append(build)
            return thunks, amts

        # ---- group chunk pipeline ----
        def run_group(g, amts, pump, last=False):
            kbs = [KBS[r][4 * g + j] for j in range(4)]
            cmax = kbs[3]

            outt = outtpool.tile([128, 512], F32, tag="outtps")
            rsps = rspool.tile([1, 512], F32, tag="rsps")

            def emit_score(c):
                jmin = min(j for j in range(4) if kbs[j] >= c)
                lo = 128 * jmin
                klo = 16 * (c % 8)
                xps = xpool.tile([128, 512], F32, tag="xps")
                nc.tensor.matmul(xps[:, lo:512], kt[:, 128 * c:128 * c + 128],
                                 qts[:, 512 * g + lo:512 * g + 512],
                                 start=True, stop=False)
                if g == 0:
                    # group A: exact-f32 A-rep then exact rank-1 negm, so
                    # fl(fl(S + F) + negm) matches the reference bit-level
                    # quantization (F magnitudes up to 1e9 here quantize S)
                    nc.tensor.matmul(xps[:, lo:512],
                                     reps[c % 8][0:klo + 16, :],
                                     amtf[0:klo + 16, lo:512],
                                     start=False, stop=False,
                                     skip_group_check=True)
                    nc.tensor.matmul(xps[:, lo:512], onesrow[:],
                                     negmrow4[0:1, lo:512],
                                     start=False, stop=True,
                                     skip_group_check=True)
                else:
                    nc.tensor.matmul(xps[:, lo:512],
                                     repsr[c % 8][0:klo + 16, :],
                                     amts[c // 8][0:klo + 16, lo:512],
                                     start=False, stop=True,
                                     skip_group_check=True)
                pts = ptspool.tile([128, 512], F32R, tag="pts")
                nc.scalar.activation(pts[:, lo:512], xps[:, lo:512], ACTF.Exp)
                if c in kbs:
                    j = kbs.index(c)
                    # zero strictly-above-diagonal (k > q); for the special
                    # block the C dust there is re-injected via cbm matmuls
                    nc.gpsimd.affine_select(
                        out=pts[:, 128 * j:128 * j + 128],
                        in_=pts[:, 128 * j:128 * j + 128],
                        pattern=[[1, 128]], base=0, channel_multiplier=-1,
                        compare_op=ALU.is_ge, fill=0.0)
                if dbg is not None and g == 0 and c == 0:
                    nc.sync.dma_start(out=dbg["PTS0"], in_=pts[:])
                return c, lo, pts

            def emit_accum(st):
                # stop whenever some block's columns see their last chunk, so
                # that block's tail (injects/den/epilogue) can read its psum
                # region while the rest keeps accumulating (swp pattern)
                c, lo, pts = st
                stops = (c == cmax) or (last and c in kbs)
                nc.tensor.matmul(rsps[0:1, lo:512], onesr[:], pts[:, lo:512],
                                 start=(c == 0), stop=stops,
                                 skip_group_check=(c != 0))
                nc.tensor.matmul(outt[:, lo:512],
                                 vsb[:, 128 * c:128 * c + 128],
                                 pts[:, lo:512],
                                 start=(c == 0), stop=stops,
                                 skip_group_check=(c != 0))

            def block_tail(j):
                """Emit block j's suffix injects, denominator, O^T->O
                transpose, 1/den scale, and output DMA; valid as soon as
                chunk kbs[j]'s accumulation has stopped."""
                b = 4 * g + j
                nsuf = T - 128 * (kbs[j] + 1)
                if kbs[j] + 1 <= CH - 1:
                    nc.tensor.matmul(
                        outt[:, 128 * j:128 * j + 128],
                        swall[0:1, 128 * b:128 * b + 128],
                        call[0:1, 128 * b:128 * b + 128],
                        start=False, stop=True, skip_group_check=True,
                    )
                if g == 0 and j == 0:
                    # block 0's above-diagonal C dust (desperate rows)
                    dc = kbs[0]
                    nc.tensor.matmul(rsps[0:1, 0:128], onesr[:], cbm[:],
                                     start=False, stop=True,
                                     skip_group_check=True)
                    nc.tensor.matmul(outt[:, 0:128],
                                     vsb[:, 128 * dc:128 * dc + 128], cbm[:],
                                     start=False, stop=True,
                                     skip_group_check=True)
                rs_sb = tinypool.tile([1, 128], F32, tag="rssb")
                nc.scalar.copy(rs_sb[:], rsps[0:1, 128 * j:128 * j + 128])
                nc.tensor.transpose(rst_r, rs_sb[:], ident[0:1, 0:1])
                den = tinypool.tile([128, 1], F32, tag="den")
                if nsuf > 0:
                    nc.vector.scalar_tensor_tensor(
                        out=den[:], in0=cees[b][:], scalar=float(nsuf),
                        in1=rst_r, op0=ALU.mult, op1=ALU.add)
                else:
                    nc.vector.tensor_copy(den[:], rst_r)
                rsum = smallpool.tile([128, 1], F32, tag="rsum")
                nc.vector.reciprocal(rsum[:], den[:])
                outt_sb = outtsbpool.tile([128, 128], F32, tag="outtsb")
                nc.scalar.copy(outt_sb[:], outt[:, 128 * j:128 * j + 128])
                ops = ptpool.tile([128, 512], F32, tag="ptps")
                nc.tensor.transpose(ops[:, 0:128], outt_sb[:], ident[:])
                nc.scalar.mul(osb[:, 128 * b:128 * b + 128], ops[:, 0:128],
                              rsum[:])
                row0 = 128 * b
                nc.sync.dma_start(out=Od[row0:row0 + 128, :],
                                  in_=osb[:, 128 * b:128 * b + 128])

            def group_tail():
                """Whole-group epilogue (non-last groups): fewer, wider ACT
                ops than four block tails."""
                for j in range(4):
                    b = 4 * g + j
                    if kbs[j] + 1 <= CH - 1:
                        nc.tensor.matmul(
                            outt[:, 128 * j:128 * j + 128],
                            swall[0:1, 128 * b:128 * b + 128],
                            call[0:1, 128 * b:128 * b + 128],
                            start=False, stop=True, skip_group_check=True,
                        )
                if g == 0:
                    dc = kbs[0]
                    nc.tensor.matmul(rsps[0:1, 0:128], onesr[:], cbm[:],
                                     start=False, stop=True,
                                     skip_group_check=True)
                    nc.tensor.matmul(outt[:, 0:128],
                                     vsb[:, 128 * dc:128 * dc + 128], cbm[:],
                                     start=False, stop=True,
                                     skip_group_check=True)
                rs_sb = tinypool.tile([1, 512], F32, tag="rssbw")
                nc.scalar.copy(rs_sb[:], rsps[:])
                rsums = []
                for j in range(4):
                    b = 4 * g + j
                    nsuf = T - 128 * (kbs[j] + 1)
                    nc.tensor.transpose(rst_r, rs_sb[0:1, 128 * j:128 * j + 128],
                                        ident[0:1, 0:1])
                    den = tinypool.tile([128, 1], F32, tag="den")
                    if nsuf > 0:
                        nc.vector.scalar_tensor_tensor(
                            out=den[:], in0=cees[b][:], scalar=float(nsuf),
                            in1=rst_r, op0=ALU.mult, op1=ALU.add)
                    else:
                        nc.vector.tensor_copy(den[:], rst_r)
                    rsum = smallpool.tile([128, 1], F32, tag="rsum")
                    nc.vector.reciprocal(rsum[:], den[:])
                    rsums.append(rsum)
                outt_sb = outtsbpool.tile([128, 512], F32, tag="outtsbw")
                nc.scalar.copy(outt_sb[:], outt[:])
                ops = ptpool.tile([128, 512], F32, tag="ptps")
                for j in range(4):
                    nc.tensor.transpose(
                        ops[:, 128 * j:128 * j + 128],
                        outt_sb[:, 128 * j:128 * j + 128], ident[:])
                for j in range(4):
                    b = 4 * g + j
                    nc.scalar.mul(osb[:, 128 * b:128 * b + 128],
                                  ops[:, 128 * j:128 * j + 128], rsums[j][:])
                    row0 = 128 * b
                    nc.sync.dma_start(out=Od[row0:row0 + 128, :],
                                      in_=osb[:, 128 * b:128 * b + 128])

            # software-pipelined emission, 2 chunks deep: S/A/exp of chunk c
            # go ahead of rowsum/PV of chunk c-2, so the in-order PE queue
            # has two chunks of slack against ACT exp jitter.  Block tails
            # are emitted as soon as their accumulation stops.  `pump` thunks
            # (later groups' mask chains / amt builds) are spread through the
            # back half of the chunk stream.
            nch = cmax + 1
            pumped = 0
            pend = []

            def after_accum(st):
                # early per-block tails only for the last group (drains the
                # program tail); elsewhere they would steal ACT slots from
                # the exp stream, which is near-critical mid-schedule
                c = st[0]
                if last and c in kbs:
                    block_tail(kbs.index(c))

            for c in range(nch):
                pend.append(emit_score(c))
                if len(pend) > 2:
                    st = pend.pop(0)
                    emit_accum(st)
                    after_accum(st)
                want = (len(pump) * (2 * (c + 1) - nch)) // max(1, nch) \
                    if 2 * (c + 1) > nch else 0
                while pumped < min(want, len(pump)):
                    pump[pumped]()
                    pumped += 1
            for st in pend:
                emit_accum(st)
                after_accum(st)
            if not last:
                group_tail()
            while pumped < len(pump):
                pump[pumped]()
                pumped += 1

        # ---- orchestration ----
        # tile-0 chains pipeline together right after topk0; topk1's serial
        # DVE rounds queue behind them (emitted after the tile-0 chain DVE
        # steps, overlapping the gA/gB pipelines whose tile-0 blocks carry
        # most of the PE work); tile-1 chains + amt builds are pumped into
        # the gB/gC chunk streams.
        gA, gB, gC, gD = GORDERS[r]
        blocksAB = [4 * gA + j for j in range(4)] + [4 * gB + j for j in range(4)]
        sds_mm(1)
        for t in interleave_chains(blocksAB):
            t()
        thA, amtsA = amt_steps(gA)
        for t in thA:
            t()
        topk_rounds(1)
        thB, amtsB = amt_steps(gB)
        run_group(gA, amtsA, pump=thB)
        chainsC = interleave_chains([4 * gC + j for j in range(4)])
        thC, amtsC = amt_steps(gC)
        run_group(gB, amtsB, pump=chainsC + thC)
        chainsD = interleave_chains([4 * gD + j for j in range(4)])
        thD, amtsD = amt_steps(gD)
        run_group(gC, amtsC, pump=chainsD + thD)
        run_group(gD, amtsD, pump=[], last=True)


_PROGRAMS = {}


def build_program(r: int, debug=False):
    key = (r, debug)
    if key in _PROGRAMS:
        return _PROGRAMS[key]
    nc = bacc.Bacc("TRN2", target_bir_lowering=False, debug=False)
    Qd = nc.dram_tensor("Q", [NLQ, HD], F32R, kind="ExternalInput").ap()
    Kd = nc.dram_tensor("K", [T, HD], F32R, kind="ExternalInput").ap()
    QDSd = nc.dram_tensor("QDS", [NLQ, HD], F32, kind="ExternalInput").ap()
    KDSd = nc.dram_tensor("KDS", [T, HD], F32, kind="ExternalInput").ap()
    Vd = nc.dram_tensor("V", [T, HD], F32R, kind="ExternalInput").ap()
    Ud = nc.dram_tensor("UBT", [128, NB], F32, kind="ExternalInput").ap()
    Od = nc.dram_tensor("OUT", [NLQ, HD], F32, kind="ExternalOutput").ap()
    dbg = None
    if debug:
        dbg = {}
        for nm, shp in (("SDS0", [128, TDS + 1]), ("SDS1", [128, TDS + 1]),
                        ("E0", [128, TDS]), ("F0", [128, TDS]),
                        ("AM0", [128, TDS]), ("PTS0", [128, 512])):
            dbg[nm] = nc.dram_tensor(nm, shp, F32, kind="ExternalOutput").ap()
    with tile.TileContext(nc) as tc:
        _kernel_body(tc, r, Qd, Kd, QDSd, KDSd, Vd, Ud, Od, dbg)
    nc.compile()
    _PROGRAMS[key] = nc
    return nc


def shard_inputs(Q, K, V, U):
    """Per-core input dicts: core = 4*r + h (devices 0-3 parity 0)."""
    maps = []
    Qr = Q[0].reshape(NH, 2 * NB, QPB, HD)
    Ur = U[0].reshape(2 * NB, QPB)
    for r in range(2):
        for h in range(NH):
            qsh = np.ascontiguousarray(Qr[h, KBS[r]].reshape(NLQ, HD))
            ubt = np.ascontiguousarray(Ur[KBS[r]].T)
            ksh = np.ascontiguousarray(K[0, h])
            maps.append({
                "Q": qsh,
                "QDS": qsh,
                "K": ksh,
                "KDS": ksh,
                "V": np.ascontiguousarray(V[0, h]),
                "UBT": ubt,
            })
    return maps


def unshard_output(outs):
    O = np.empty((B, NH, T, HD), np.float32)
    Ov = O[0].reshape(NH, 2 * NB, QPB, HD)
    i = 0
    for r in range(2):
        for h in range(NH):
            Ov[h, KBS[r]] = outs[i]["OUT"].reshape(NB, QPB, HD)
            i += 1
    return O


def _run_concurrent(in_maps):
    """Dispatch parity-0 on devices 0-3 and parity-1 on devices 4-7."""
    import jax
    from jax.sharding import Mesh, PartitionSpec
    from jax.experimental.shard_map import shard_map
    from concourse import bass2jax

    bass2jax.install_neuronx_cc_hook()
    devices = jax.devices()
    assert len(devices) >= 8, f"need 8 neuron cores, got {len(devices)}"

    pending = []
    for r in range(2):
        nc = build_program(r)
        maps = in_maps[4 * r:4 * r + 4]
        pname = nc.partition_id_tensor.name if nc.partition_id_tensor else None
        in_names, out_names, out_avals, zero_outs = [], [], [], []
        for alloc in nc.m.functions[0].allocations:
            if not isinstance(alloc, mybir.MemoryLocationSet):
                continue
            name = alloc.memorylocations[0].name
            if alloc.kind == "ExternalInput":
                if name != pname:
                    in_names.append(name)
            elif alloc.kind == "ExternalOutput":
                out_names.append(name)
                shape = tuple(alloc.tensor_shape)
                dtype = mybir.dt.np(alloc.dtype)
                out_avals.append(jax.core.ShapedArray(shape, dtype))
                zero_outs.append(np.zeros(shape, dtype))
        n_params = len(in_names)
        n_outs = len(out_avals)
        all_names = in_names + out_names
        if pname is not None:
            all_names = all_names + [pname]
        donate = tuple(range(n_params, n_params + n_outs))

        def _body(*args, _nc=nc, _avals=tuple(out_avals),
                  _names=tuple(all_names), _onames=tuple(out_names),
                  _pname=pname):
            operands = list(args)
            if _pname is not None:
                operands.append(bass2jax.partition_id_tensor())
            outs = bass2jax._bass_exec_p.bind(
                *operands,
                out_avals=_avals,
                in_names=_names,
                out_names=_onames,
                lowering_input_output_aliases=(),
                sim_require_finite=True,
                sim_require_nnan=True,
                nc=_nc,
            )
            return tuple(outs)

        mesh = Mesh(np.asarray(devices[4 * r:4 * r + 4]), ("core",))
        in_specs = (PartitionSpec("core"),) * (n_params + n_outs)
        out_specs = (PartitionSpec("core"),) * n_outs
        fn = jax.jit(
            shard_map(_body, mesh=mesh, in_specs=in_specs,
                      out_specs=out_specs, check_rep=False),
            donate_argnums=donate, keep_unused=True,
        )
        per_core = [[np.asarray(m[nm]) for nm in in_names] for m in maps]
        concat_in = [
            np.concatenate([per_core[c][i] for c in range(4)], axis=0)
            for i in range(n_params)
        ]
        concat_zero = [
            np.concatenate([z] * 4, axis=0) for z in zero_outs
        ]
        out_arrs = fn(*concat_in, *concat_zero)
        pending.append((out_arrs, out_names))

    results = []
    for r, (out_arrs, out_names) in enumerate(pending):
        outs = [np.asarray(a) for a in out_arrs]
        for c in range(4):
            d = {}
            for i, nm in enumerate(out_names):
                n0 = outs[i].shape[0] // 4
                d[nm] = outs[i][c * n0:(c + 1) * n0]
            results.append(d)
    return results


def kernel(**inputs):
    Q = np.asarray(inputs["Q"], np.float32)
    K = np.asarray(inputs["K"], np.float32)
    V = np.asarray(inputs["V"], np.float32)
    U = np.asarray(inputs["U"], np.float32)
    in_maps = shard_inputs(Q, K, V, U)
    results = _run_concurrent(in_maps)
    return unshard_output(results)


# revision 6
# speedup vs baseline: 1.4404x; 1.0105x over previous
"""DSALite sparse-attention Trainium2 kernel, transposed-flow redesign.

Problem: B=1, nH=4, T=4096, hd=128 attention where the mask is derived from
8x-downsampled scores: per full row, threshold = 128th largest of the 512
downsampled (and u-scaled) scores, mask = sigmoid((s - thr) * 10 * u) * causal,
scores += (1-mask) * (-1e9), softmax, @V.

Sharding: 8 cores = 4 heads x 2 row-parities (identical to the baseline
kernel).  Core (h, r) handles head h and the 16 query blocks KBS[r].

v2 redesign (vs the q-major baseline): all per-cell work runs in the
TRANSPOSED domain X^T[k, q] so the mask add, the row-max subtraction and the
softmax denominator come out of PE matmuls instead of DVE elementwise ops:

  per 128-k chunk c, per 4-block group (512 q):
    X^T  = K_c Q^T           (fp32r matmul, scale folded into Q^T)
         + Rep(c) @ amin2T   (replicates 16 ds-mask rows onto 128 k rows;
                              amin2 = min(F, ds-causal) + negm pre-folded)
    P^T  = exp(X^T)          (one ACT op PSUM->SBUF(f32r), no bias needed)
    diag chunk: zero k>q half via one gpsimd affine_select (reference
                contributes exactly 0 there: exp(S - 1e9 - M) underflows)
    den += ones^T @ P^T      (PE rank-1 into a [1,512] accumulator)
    O^T += V_c^T @ P^T       (fp32r accumulation, 512 wide)

This deletes the baseline's three big serial DVE/ACT burdens: the X = S+A
elementwise add (51us DVE), the P chunk transposes (34k PE cycles), and the
PSUM->SBUF P^T copies (~20us DVE + ACT).  The mask chain itself (exact f32
sigmoid bucket semantics, top-128 threshold via DVE max8/match_replace) is
carried over op-for-op from the baseline; amin2T is produced by 40 small PE
transposes of the per-block amin2 = amin + negm tiles.

Numerical notes (same bucket-exactness strategy as the baseline):
  - ds scores/threshold/sigmoid/F chain identical to baseline (exact f32).
  - X^T accumulates S~fp32r + amin2 + negm in f32 PSUM adds; only the smooth
    softmax path sees the fp32r rounding, mask buckets are computed exactly.
  - denominator comes from ones@P^T in fp32r (P in [0,1], err ~2^-21 rel).
  - suffix (fully-masked) columns contribute C = exp(-1e9 - M) per row via
    rank-1 SW x C^T injects and a C * n_suffix denominator fixup (baseline
    mechanism, unchanged).
"""

import numpy as np

import concourse.bass as bass
import concourse.bacc as bacc
import concourse.mybir as mybir
import concourse.tile as tile
from concourse.masks import make_identity

F32 = mybir.dt.float32
F32R = mybir.dt.float32r
BF16 = mybir.dt.bfloat16
ALU = mybir.AluOpType
ACTF = mybir.ActivationFunctionType
AX = mybir.AxisListType

B, NH, T, HD = 1, 4, 4096, 128
STRIDE = 8
TDS = T // STRIDE          # 512 downsampled positions
KDS = 128                  # exact 128th largest per ds row
NEG = -1e9
SCALE = HD ** -0.5
ZAP = -1e30

NB = 16                    # 128-row query blocks per core
QPB = 128
NLQ = NB * QPB             # 2048 local query rows
NG = 4                     # groups of 4 blocks (512 q)
CH = T // 128              # 32 key/value chunks
CMW = 1008                 # sliding ds-causal const width
GORDERS = {0: [0, 1, 2, 3], 1: [0, 1, 2, 3]}
# Same per-program block SETS as the baseline (near-equal causal work), but
# ordered so sds-tile 0 (list positions 0-7 = groups A,B) carries the bulk of
# the PE work: group B's big blocks keep PE busy for the ~19us that tile 1's
# serial top-k chain occupies the DVE.  Groups ascend within themselves
# (the jmin narrowing logic requires it); block 0 of the list must be the
# program's earliest block (desperate-row special handling).
KBS = [
    [0, 2, 4, 6, 24, 27, 29, 31, 8, 10, 12, 14, 18, 20, 22, 23],
    [1, 3, 5, 7, 25, 26, 28, 30, 9, 11, 13, 15, 16, 17, 19, 21],
]


def _consts(nc, pool):
    ident = pool.tile([128, 128], F32, tag="ident")
    make_identity(nc, ident[:])
    identr = pool.tile([128, 128], F32R, tag="identr")
    nc.vector.tensor_copy(identr[:], ident[:])

    # cm3[i, jj] = 0.0 if jj <= 496 + i//8 else -1e9 (sliding ds-causal mask)
    cm3 = pool.tile([128, CMW], F32, tag="cm3")
    nc.gpsimd.memset(cm3[:], 0.0)
    nc.gpsimd.affine_select(
        out=cm3[:], in_=cm3[:], pattern=[[-8, CMW]], base=3968,
        channel_multiplier=1, compare_op=ALU.is_ge, fill=NEG,
    )

    # rep[bp][k, i] = 1.0 iff k == 16*bp + i//8 (f32 for the mask chain,
    # f32r copies for the X^T mask-replication matmuls)
    reps, repsr = [], []
    for bp in range(8):
        rep = pool.tile([128, 128], F32, tag=f"rep{bp}")
        nc.gpsimd.memset(rep[:], 1.0)
        nc.gpsimd.affine_select(
            out=rep[:], in_=rep[:], pattern=[[1, 128]], base=128 * bp,
            channel_multiplier=-8, compare_op=ALU.is_ge, fill=0.0)
        nc.gpsimd.affine_select(
            out=rep[:], in_=rep[:], pattern=[[-1, 128]], base=7 - 128 * bp,
            channel_multiplier=8, compare_op=ALU.is_ge, fill=0.0)
        reps.append(rep)
        repr_ = pool.tile([128, 128], F32R, tag=f"repr{bp}")
        nc.vector.tensor_copy(repr_[:], rep[:])
        repsr.append(repr_)

    onesf = pool.tile([128, 1], F32, tag="onesf")
    nc.gpsimd.memset(onesf[:], 1.0)
    onesr = pool.tile([128, 1], F32R, tag="onesr")
    nc.vector.tensor_copy(onesr[:], onesf[:])
    onesb = pool.tile([128, 1], BF16, tag="onesb")
    nc.gpsimd.memset(onesb[:], 1.0)
    onesrow = pool.tile([1, 128], F32, tag="onesrow")
    nc.gpsimd.memset(onesrow[:], 1.0)

    # c01T[i, j] = 1 where i > j (strictly below diagonal in [k, q] layout =
    # above-diagonal in q-major): the region of the special block's diagonal
    # chunk overwritten with the per-row masked constant C.
    c01t = pool.tile([128, 128], mybir.dt.int8, tag="c01t")
    nc.gpsimd.memset(c01t[:], 1)
    nc.gpsimd.affine_select(
        out=c01t[:], in_=c01t[:], pattern=[[-1, 128]], base=-1,
        channel_multiplier=1, compare_op=ALU.is_ge, fill=0,
    )

    return ident, identr, cm3, reps, repsr, onesr, onesb, onesrow, c01t


def _kernel_body(tc, r, Qd, Kd, QDSd, KDSd, Vd, Ud, Od, dbg=None):
    nc = tc.nc
    from contextlib import ExitStack
    with ExitStack() as ctx:
        cpool = ctx.enter_context(tc.tile_pool(name="consts", bufs=1))
        inpool = ctx.enter_context(tc.tile_pool(name="inputs", bufs=1))
        # PSUM budget (8 banks): xps 2 + outt 1 + rsps 1 + maskps 2 + ptps 1
        # + scratch 1
        xpool = ctx.enter_context(tc.tile_pool(name="xps", bufs=2, space="PSUM"))
        outtpool = ctx.enter_context(tc.tile_pool(name="outtps", bufs=1, space="PSUM"))
        rspool = ctx.enter_context(tc.tile_pool(name="rsps", bufs=1, space="PSUM"))
        mpspool = ctx.enter_context(tc.tile_pool(name="maskps", bufs=2, space="PSUM"))
        ptpool = ctx.enter_context(tc.tile_pool(name="ptps", bufs=1, space="PSUM"))
        scrpspool = ctx.enter_context(tc.tile_pool(name="scrps", bufs=1, space="PSUM"))
        sdspool = ctx.enter_context(tc.tile_pool(name="sds", bufs=1))
        scrpool = ctx.enter_context(tc.tile_pool(name="scr", bufs=1))
        epool = ctx.enter_context(tc.tile_pool(name="e", bufs=4))
        fpool = ctx.enter_context(tc.tile_pool(name="f", bufs=5))
        aminpool = ctx.enter_context(tc.tile_pool(name="amin", bufs=3))
        am2pool = ctx.enter_context(tc.tile_pool(name="am2", bufs=8))
        amtpool = ctx.enter_context(tc.tile_pool(name="amt", bufs=7))
        smallpool = ctx.enter_context(tc.tile_pool(name="small", bufs=NB))
        tinypool = ctx.enter_context(tc.tile_pool(name="tiny", bufs=6))
        ptspool = ctx.enter_context(tc.tile_pool(name="pts", bufs=4))
        outtsbpool = ctx.enter_context(tc.tile_pool(name="outtsb", bufs=3))
        swsbpool = ctx.enter_context(tc.tile_pool(name="swsb", bufs=1))

        (ident, identr, cm3, reps, repsr, onesr, onesb, onesrow,
         c01t) = _consts(nc, cpool)

        # one shared PSUM scratch bank; disjoint regions, subtile-dep tracked
        scrps = scrpspool.tile([128, 512], F32, tag="scrps")
        swp = scrps[0:1, 0:128]        # suffix colsum accumulator
        ps1b_r = scrps[0:128, 128:129]  # threshold replicate matmul out
        ctp_r = scrps[0:1, 192:320]     # cee^T transpose out
        rst_r = scrps[0:128, 352:353]   # rowsum^T transpose out
        ngt_r = scrps[0:1, 384:512]     # negm^T transpose out (block 0)

        # ---- loads: ds subsets first (they gate the serial top-k chain) ----
        kt = inpool.tile([128, T], F32R, tag="kt")     # K^T [d, t]
        qts = inpool.tile([128, NLQ], F32R, tag="qts")  # Q^T * scale [d, q]
        kdst = inpool.tile([128, TDS], F32, tag="kdst")
        qdst = inpool.tile([128, TDS // 2], F32, tag="qdst")
        prep = ctx.enter_context(tc.tile_pool(name="prep", bufs=2))
        for dsrc, dsdst, nds in ((KDSd, kdst, TDS), (QDSd, qdst, TDS // 2)):
            natd = prep.tile([128, nds], F32, tag="natd", name="natd")
            nc.sync.dma_start(
                out=natd[:].rearrange("p (c d) -> p c d", d=128),
                in_=dsrc.rearrange("(c p s) d -> p c s d", p=128,
                                   s=STRIDE)[:, :, 0, :],
            )
            n3 = natd[:].rearrange("p (c d) -> p c d", d=128)
            ptd = ptpool.tile([128, 512], F32, tag="ptps", name="ppsd")
            for j in range(nds // 128):
                nc.tensor.transpose(
                    ptd[:, 128 * j:128 * j + 128], n3[:, j, :], ident[:])
            # ACT copy: keeps the DVE queue free so topk0 starts immediately
            nc.scalar.copy(dsdst[:], ptd[:, 0:nds])

        ub = inpool.tile([128, NB], F32, tag="ub")
        nc.sync.dma_start(out=ub[:], in_=Ud[:])
        usc = inpool.tile([128, NB], F32, tag="usc")
        nc.vector.tensor_scalar(usc[:], ub[:], 0.0, 1.0, op0=ALU.max,
                                op1=ALU.min)
        nc.vector.tensor_scalar(usc[:], usc[:], 1.0, None, op0=ALU.add)
        vsb = inpool.tile([128, T], F32R, tag="vsb")   # [t, d] natural

        def v_load():
            # V early: the suffix colsums are the main PE work available
            # under the serial topk0 DVE chain
            nc.sync.dma_start(
                out=vsb[:].rearrange("p (c d) -> p c d", d=128),
                in_=Vd.rearrange("(c p) d -> p c d", p=128),
            )

        def full_prep():
            # K^T: 8 pieces of 512 cols; f32r transposes (1.5 cyc/row)
            s3 = Kd.rearrange("(c p) d -> p c d", p=128)
            for c4 in range(8):
                nat = prep.tile([128, 512], F32R, tag="nat")
                nc.sync.dma_start(
                    out=nat[:].rearrange("p (c d) -> p c d", d=128),
                    in_=s3[:, 4 * c4:4 * c4 + 4, :],
                )
                n3 = nat[:].rearrange("p (c d) -> p c d", d=128)
                pt = ptpool.tile([128, 512], F32R, tag="ptps")
                for j in range(4):
                    nc.tensor.transpose(
                        pt[:, 128 * j:128 * j + 128], n3[:, j, :], identr[:])
                nc.any.tensor_copy(kt[:, 512 * c4:512 * c4 + 512], pt[:])
            # Q^T with the score scale folded into the PSUM->SBUF copy
            s3 = Qd.rearrange("(c p) d -> p c d", p=128)
            for c4 in range(4):
                nat = prep.tile([128, 512], F32R, tag="nat")
                nc.sync.dma_start(
                    out=nat[:].rearrange("p (c d) -> p c d", d=128),
                    in_=s3[:, 4 * c4:4 * c4 + 4, :],
                )
                n3 = nat[:].rearrange("p (c d) -> p c d", d=128)
                pt = ptpool.tile([128, 512], F32R, tag="ptps")
                for j in range(4):
                    nc.tensor.transpose(
                        pt[:, 128 * j:128 * j + 128], n3[:, j, :], identr[:])
                nc.scalar.mul(qts[:, 512 * c4:512 * c4 + 512], pt[:], SCALE)

        # ---- downsampled scores + exact per-row 128th largest ----
        # Split so sds matmuls (PE) can be emitted early while the serial
        # DVE top-k rounds are placed where the DVE queue wants them.
        sds_tiles = {}

        def sds_mm(t):
            sds = sdspool.tile([128, TDS + 1], F32, tag=f"sds{t}",
                               name=f"sds{t}")
            sds_tiles[t] = sds
            ps = mpspool.tile([128, 512], F32, tag="maskps", name=f"sdsps{t}")
            nc.tensor.matmul(ps[:], qdst[:, 128 * t:128 * t + 128], kdst[:])
            nc.scalar.mul(sds[:, 0:TDS], ps[:], SCALE)

        def topk_rounds(t):
            sds = sds_tiles[t]
            scr = scrpool.tile([128, TDS], F32, tag="scr", name=f"scr{t}")
            nc.vector.tensor_copy(scr[:], sds[:, 0:TDS])
            maxsc = scrpool.tile([128, 8], F32, tag="maxsc", name=f"maxsc{t}")
            for rnd in range(KDS // 8):
                nc.vector.max(out=maxsc[:], in_=scr[:])
                if rnd < KDS // 8 - 1:
                    nc.vector.match_replace(
                        out=scr[:], in_to_replace=maxsc[:], in_values=scr[:],
                        imm_value=ZAP,
                    )
            nc.vector.tensor_copy(sds[:, TDS:TDS + 1], maxsc[:, 7:8])
            if dbg is not None:
                nc.sync.dma_start(out=dbg[f"SDS{t}"], in_=sds[:])

        sds_mm(0)
        topk_rounds(0)
        v_load()
        full_prep()

        # ---- suffix V column-sums SW(cb), [1, 128] slices in swall ----
        # bf16 V copy makes the narrow [1,128]-out colsum matmuls 1 cyc/row
        # (fp32r pays a 4x penalty below 256-wide); SW only feeds desperate
        # rows via C = exp(-1e9 - M), so 0.4% bf16 rounding is irrelevant.
        osb = inpool.tile([128, NB * 128], F32, tag="osb")
        vsbb = inpool.tile([128, T], BF16, tag="vsbb")
        nc.gpsimd.tensor_copy(vsbb[:], vsb[:])
        swall = swsbpool.tile([1, NB * 128], BF16, tag="swall")
        call = swsbpool.tile([1, NB * 128], BF16, tag="call")
        negmrow4 = swsbpool.tile([1, 512], F32, tag="negmrow4")
        ceerow0 = swsbpool.tile([1, 128], F32, tag="ceerow0")
        amtf = inpool.tile([128, 512], F32, tag="amtf")   # gA amin^T (f32)
        # cbm[k, q] = C(q) for k > q else 0: the diag chunk's above-diagonal
        # dust of the desperate-capable block, injected via two rank-128
        # matmuls into rsps and outt (pts itself stays affine-zeroed there)
        cbm = inpool.tile([128, 128], F32R, tag="cbm")
        emitted = 0
        prev = CH
        for b in sorted(range(NB), key=lambda b_: KBS[r][b_], reverse=True):
            cb = KBS[r][b] + 1
            for c in range(cb, prev):
                emitted += 1
                nc.tensor.matmul(swp, onesb[:], vsbb[:, 128 * c:128 * c + 128],
                                 start=(emitted == 1), stop=(c == cb),
                                 skip_group_check=(emitted != 1))
            prev = cb
            if cb <= CH - 1:
                nc.scalar.copy(swall[0:1, 128 * b:128 * b + 128], swp)

        # ---- mask chain (q-major, bucket-exact; -> amin2 f32r + cee) ----
        # Emitted as per-block STEP thunks so 8 blocks' chains interleave
        # (pipelining the 12-step cross-engine latency chain) and so steps can
        # be pumped into the chunk pipelines of earlier groups.
        negms, cees, am2s = {}, {}, {}

        def chain_steps(b):
            kb = KBS[r][b]
            nd = 16 * (kb + 1)
            tt, pp = divmod(b, 8)
            st = {}

            def s0():
                sds = sds_tiles[tt]
                st["ps"] = mpspool.tile([128, 512], F32, tag="maskps", name="chps")
                nc.tensor.matmul(st["ps"][:, 0:nd], reps[pp][:], sds[:, 0:nd])
                nc.tensor.matmul(ps1b_r, reps[pp][:], sds[:, TDS:TDS + 1])
                st["thru"] = tinypool.tile([128, 1], F32, tag="thru", name="thru")
                nc.vector.tensor_scalar(st["thru"][:], ps1b_r, usc[:, b:b + 1],
                                        None, op0=ALU.mult)

            def s1():
                # m = 1/(1 + exp(-10*arg)); bucket-exact vs XLA f32 sigmoid
                st["arg"] = epool.tile([128, TDS], F32, tag="arg", name="arg")
                nc.vector.tensor_scalar(st["arg"][:, 0:nd], st["ps"][:, 0:nd],
                                        usc[:, b:b + 1], st["thru"][:],
                                        op0=ALU.mult, op1=ALU.subtract)

            def s2():
                st["z"] = epool.tile([128, TDS], F32, tag="z", name="z")
                nc.scalar.activation(st["z"][:, 0:nd], st["arg"][:, 0:nd],
                                     ACTF.Exp, scale=-10.0)

            def s3():
                nc.gpsimd.tensor_scalar(st["z"][:, 0:nd], st["z"][:, 0:nd],
                                        1.0, None, op0=ALU.add)

            def s4():
                st["e"] = epool.tile([128, TDS], F32, tag="e", name="e")
                nc.vector.reciprocal(st["e"][:, 0:nd], st["z"][:, 0:nd])

            def s5():
                st["f"] = fpool.tile([128, TDS], F32, tag="f", name="f")
                nc.gpsimd.tensor_scalar(st["f"][:, 0:nd], st["e"][:, 0:nd],
                                        -1.0, 1.0, op0=ALU.mult, op1=ALU.add)

            def s6():
                nc.gpsimd.tensor_scalar(st["f"][:, 0:nd], st["f"][:, 0:nd],
                                        NEG, None, op0=ALU.mult)

            def s7():
                # amin = min(F, ds-causal): F <= 0 always and cm3 is 0 on all
                # fully-allowed columns, so the min only matters on the last
                # 16 ds cols (the diagonal ds band, cm3[:, 496:512]) -- apply
                # it in place on f, 16 cols instead of nd.
                nc.vector.tensor_tensor(st["f"][:, nd - 16:nd],
                                        st["f"][:, nd - 16:nd],
                                        cm3[:, 496:512], op=ALU.min)
                st["amin"] = st["f"]

            def s8():
                negm = smallpool.tile([128, 1], F32, tag="negm")
                nc.vector.tensor_reduce(negm[:], st["amin"][:, 0:nd], axis=AX.X,
                                        op=ALU.max, negate=True)
                negms[b] = negm

            def s9():
                if b < 4:
                    # group A blocks can hold desperate or SEMI-desperate
                    # rows (|max F| large): the reference's fl(S + F)
                    # quantizes S at ulp(F), so F must stay UNFOLDED in
                    # exact f32 and negm must be added only after the S+F
                    # rounding (fl(fl(S+F) + negm), matching the ref)
                    amu = am2pool.tile([128, TDS], F32, tag="am2", name="amu")
                    nc.vector.tensor_copy(amu[:, 0:nd], st["amin"][:, 0:nd])
                    am2s[b] = amu
                    return
                # amin2 = amin + negm (X^T = S + amin2 needs no later bias);
                # safe here: every masked column of these blocks has C == 0
                am2 = am2pool.tile([128, TDS], F32R, tag="am2", name="am2")
                nc.gpsimd.tensor_scalar(am2[:, 0:nd], st["amin"][:, 0:nd],
                                        negms[b][:], None, op0=ALU.add)
                am2s[b] = am2

            def s10():
                b2 = tinypool.tile([128, 1], F32, tag="b2")
                nc.vector.tensor_scalar(b2[:], negms[b][:], NEG, None,
                                        op0=ALU.add)
                cee = smallpool.tile([128, 1], F32, tag="cee")
                nc.scalar.activation(cee[:], b2[:], ACTF.Exp)
                cees[b] = cee

            def s11():
                kb_ = KBS[r][b]
                if kb_ + 1 <= CH - 1:
                    nc.tensor.transpose(ctp_r, cees[b][:], ident[:])
                    nc.scalar.copy(call[0:1, 128 * b:128 * b + 128], ctp_r)
                if b < 4:
                    # f32 rows for the exact rank-1 negm inject (+ C bcast)
                    if b == 0:
                        nc.scalar.copy(ceerow0[:], ctp_r)
                    nc.tensor.transpose(ngt_r, negms[b][:], ident[:])
                    nc.scalar.copy(negmrow4[0:1, 128 * b:128 * b + 128],
                                   ngt_r)
                if dbg is not None and b == 0:
                    nc.sync.dma_start(out=dbg["E0"], in_=st["e"][:])
                    nc.sync.dma_start(out=dbg["AM0"], in_=st["amin"][:])

            return [s0, s1, s2, s3, s4, s5, s6, s7, s8, s9, s10, s11]

        def interleave_chains(blocks):
            """Wavefront-interleaved chain thunks: chain i runs one step
            behind chain i-1, so pool rotation stays alloc-after-consumer
            and every wave packs different engines."""
            chains = [chain_steps(b) for b in blocks]
            out = []
            for w in range(len(chains) + 11):
                for i, ch in enumerate(chains):
                    s = w - i
                    if 0 <= s < 12:
                        out.append(ch[s])
            return out

        def amt_steps(g):
            """amin2T piece-build thunks for group g; returns (thunks, amts)."""
            kbs = [KBS[r][4 * g + j] for j in range(4)]
            cmax = kbs[3]
            npieces = (cmax + 1 + 7) // 8
            amts = []
            thunks = []
            if g == 0:
                # group A: one exact-f32 amin^T tile for all four blocks
                # (no folded pieces) + block 0's masked-C broadcast tile
                def build_exact(j):
                    def run():
                        b = j
                        nd = 16 * (kbs[j] + 1)
                        pt = ptpool.tile([128, 512], F32, tag="ptps",
                                         name="ptspec")
                        nc.tensor.transpose(pt[0:nd, 128 * j:128 * j + 128],
                                            am2s[b][:, 0:nd], ident[:])
                        nc.any.tensor_copy(
                            amtf[0:nd, 128 * j:128 * j + 128],
                            pt[0:nd, 128 * j:128 * j + 128])
                        if j == 0:
                            nc.tensor.matmul(pt[:, 128:256], onesrow[:],
                                             ceerow0[:])
                            nc.vector.tensor_copy(cbm[:], pt[:, 128:256])
                            nc.gpsimd.affine_select(
                                out=cbm[:], in_=cbm[:], pattern=[[-1, 128]],
                                base=-1, channel_multiplier=1,
                                compare_op=ALU.is_ge, fill=0.0)
                    return run
                for j in range(4):
                    thunks.append(build_exact(j))
                return thunks, amts
            for p in range(npieces):
                amt = amtpool.tile([128, 512], F32R, tag="amt",
                                   name=f"amt{g}_{p}")
                amts.append(amt)

                def build(p=p, amt=amt):
                    pt = ptpool.tile([128, 512], F32R, tag="ptps")
                    for j in range(4):
                        b = 4 * g + j
                        nd = 16 * (kbs[j] + 1)
                        w = min(128, nd - 128 * p)
                        if w <= 0:
                            continue
                        nc.tensor.transpose(
                            pt[0:w, 128 * j:128 * j + 128],
                            am2s[b][:, 128 * p:128 * p + w], identr[:])
                        nc.any.tensor_copy(amt[0:w, 128 * j:128 * j + 128],
                                           pt[0:w, 128 * j:128 * j + 128])
                thunks.append(build)
            return thunks, amts

        # ---- group chunk pipeline ----
        def run_group(g, amts, pump, last=False):
            kbs = [KBS[r][4 * g + j] for j in range(4)]
            cmax = kbs[3]

            outt = outtpool.tile([128, 512], F32, tag="outtps")
            rsps = rspool.tile([1, 512], F32, tag="rsps")

            def emit_score(c):
                jmin = min(j for j in range(4) if kbs[j] >= c)
                lo = 128 * jmin
                klo = 16 * (c % 8)
                xps = xpool.tile([128, 512], F32, tag="xps")
                nc.tensor.matmul(xps[:, lo:512], kt[:, 128 * c:128 * c + 128],
                                 qts[:, 512 * g + lo:512 * g + 512],
                                 start=True, stop=False)
                if g == 0:
                    # group A: exact-f32 A-rep then exact rank-1 negm, so
                    # fl(fl(S + F) + negm) matches the reference bit-level
                    # quantization (F magnitudes up to 1e9 here quantize S)
                    nc.tensor.matmul(xps[:, lo:512],
                                     reps[c % 8][0:klo + 16, :],
                                     amtf[0:klo + 16, lo:512],
                                     start=False, stop=False,
                                     skip_group_check=True)
                    nc.tensor.matmul(xps[:, lo:512], onesrow[:],
                                     negmrow4[0:1, lo:512],
                                     start=False, stop=True,
                                     skip_group_check=True)
                else:
                    nc.tensor.matmul(xps[:, lo:512],
                                     repsr[c % 8][0:klo + 16, :],
                                     amts[c // 8][0:klo + 16, lo:512],
                                     start=False, stop=True,
                                     skip_group_check=True)
                pts = ptspool.tile([128, 512], F32R, tag="pts")
                nc.scalar.activation(pts[:, lo:512], xps[:, lo:512], ACTF.Exp)
                if c in kbs:
                    j = kbs.index(c)
                    # zero strictly-above-diagonal (k > q); for the special
                    # block the C dust there is re-injected via cbm matmuls
                    nc.gpsimd.affine_select(
                        out=pts[:, 128 * j:128 * j + 128],
                        in_=pts[:, 128 * j:128 * j + 128],
                        pattern=[[1, 128]], base=0, channel_multiplier=-1,
                        compare_op=ALU.is_ge, fill=0.0)
                if dbg is not None and g == 0 and c == 0:
                    nc.sync.dma_start(out=dbg["PTS0"], in_=pts[:])
                return c, lo, pts

            def emit_accum(st):
                # stop whenever some block's columns see their last chunk, so
                # that block's tail (injects/den/epilogue) can read its psum
                # region while the rest keeps accumulating (swp pattern)
                c, lo, pts = st
                stops = (c == cmax) or (last and c in kbs)
                nc.tensor.matmul(rsps[0:1, lo:512], onesr[:], pts[:, lo:512],
                                 start=(c == 0), stop=stops,
                                 skip_group_check=(c != 0))
                nc.tensor.matmul(outt[:, lo:512],
                                 vsb[:, 128 * c:128 * c + 128],
                                 pts[:, lo:512],
                                 start=(c == 0), stop=stops,
                                 skip_group_check=(c != 0))

            def block_tail(j):
                """Emit block j's suffix injects, denominator, O^T->O
                transpose, 1/den scale, and output DMA; valid as soon as
                chunk kbs[j]'s accumulation has stopped."""
                b = 4 * g + j
                nsuf = T - 128 * (kbs[j] + 1)
                if kbs[j] + 1 <= CH - 1:
                    nc.tensor.matmul(
                        outt[:, 128 * j:128 * j + 128],
                        swall[0:1, 128 * b:128 * b + 128],
                        call[0:1, 128 * b:128 * b + 128],
                        start=False, stop=True, skip_group_check=True,
                    )
                if g == 0 and j == 0:
                    # block 0's above-diagonal C dust (desperate rows)
                    dc = kbs[0]
                    nc.tensor.matmul(rsps[0:1, 0:128], onesr[:], cbm[:],
                                     start=False, stop=True,
                                     skip_group_check=True)
                    nc.tensor.matmul(outt[:, 0:128],
                                     vsb[:, 128 * dc:128 * dc + 128], cbm[:],
                                     start=False, stop=True,
                                     skip_group_check=True)
                rs_sb = tinypool.tile([1, 128], F32, tag="rssb")
                nc.scalar.copy(rs_sb[:], rsps[0:1, 128 * j:128 * j + 128])
                nc.tensor.transpose(rst_r, rs_sb[:], ident[0:1, 0:1])
                den = tinypool.tile([128, 1], F32, tag="den")
                if nsuf > 0:
                    nc.vector.scalar_tensor_tensor(
                        out=den[:], in0=cees[b][:], scalar=float(nsuf),
                        in1=rst_r, op0=ALU.mult, op1=ALU.add)
                else:
                    nc.vector.tensor_copy(den[:], rst_r)
                rsum = smallpool.tile([128, 1], F32, tag="rsum")
                nc.vector.reciprocal(rsum[:], den[:])
                outt_sb = outtsbpool.tile([128, 128], F32, tag="outtsb")
                nc.scalar.copy(outt_sb[:], outt[:, 128 * j:128 * j + 128])
                ops = ptpool.tile([128, 512], F32, tag="ptps")
                nc.tensor.transpose(ops[:, 0:128], outt_sb[:], ident[:])
                nc.vector.tensor_scalar(osb[:, 128 * b:128 * b + 128],
                                        ops[:, 0:128], rsum[:], None,
                                        op0=ALU.mult)
                row0 = 128 * b
                nc.sync.dma_start(out=Od[row0:row0 + 128, :],
                                  in_=osb[:, 128 * b:128 * b + 128])

            def group_tail():
                """Whole-group epilogue (non-last groups): fewer, wider ACT
                ops than four block tails."""
                for j in range(4):
                    b = 4 * g + j
                    if kbs[j] + 1 <= CH - 1:
                        nc.tensor.matmul(
                            outt[:, 128 * j:128 * j + 128],
                            swall[0:1, 128 * b:128 * b + 128],
                            call[0:1, 128 * b:128 * b + 128],
                            start=False, stop=True, skip_group_check=True,
                        )
                if g == 0:
                    dc = kbs[0]
                    nc.tensor.matmul(rsps[0:1, 0:128], onesr[:], cbm[:],
                                     start=False, stop=True,
                                     skip_group_check=True)
                    nc.tensor.matmul(outt[:, 0:128],
                                     vsb[:, 128 * dc:128 * dc + 128], cbm[:],
                                     start=False, stop=True,
                                     skip_group_check=True)
                rs_sb = tinypool.tile([1, 512], F32, tag="rssbw")
                nc.scalar.copy(rs_sb[:], rsps[:])
                rsums = []
                for j in range(4):
                    b = 4 * g + j
                    nsuf = T - 128 * (kbs[j] + 1)
                    nc.tensor.transpose(rst_r, rs_sb[0:1, 128 * j:128 * j + 128],
                                        ident[0:1, 0:1])
                    den = tinypool.tile([128, 1], F32, tag="den")
                    if nsuf > 0:
                        nc.vector.scalar_tensor_tensor(
                            out=den[:], in0=cees[b][:], scalar=float(nsuf),
                            in1=rst_r, op0=ALU.mult, op1=ALU.add)
                    else:
                        nc.vector.tensor_copy(den[:], rst_r)
                    rsum = smallpool.tile([128, 1], F32, tag="rsum")
                    nc.vector.reciprocal(rsum[:], den[:])
                    rsums.append(rsum)
                outt_sb = outtsbpool.tile([128, 512], F32, tag="outtsbw")
                nc.scalar.copy(outt_sb[:], outt[:])
                ops = ptpool.tile([128, 512], F32, tag="ptps")
                for j in range(4):
                    nc.tensor.transpose(
                        ops[:, 128 * j:128 * j + 128],
                        outt_sb[:, 128 * j:128 * j + 128], ident[:])
                for j in range(4):
                    b = 4 * g + j
                    nc.vector.tensor_scalar(osb[:, 128 * b:128 * b + 128],
                                            ops[:, 128 * j:128 * j + 128],
                                            rsums[j][:], None, op0=ALU.mult)
                    row0 = 128 * b
                    nc.sync.dma_start(out=Od[row0:row0 + 128, :],
                                      in_=osb[:, 128 * b:128 * b + 128])

            # software-pipelined emission, 2 chunks deep: S/A/exp of chunk c
            # go ahead of rowsum/PV of chunk c-2, so the in-order PE queue
            # has two chunks of slack against ACT exp jitter.  Block tails
            # are emitted as soon as their accumulation stops.  `pump` thunks
            # (later groups' mask chains / amt builds) are spread through the
            # back half of the chunk stream.
            nch = cmax + 1
            pumped = 0
            pend = []

            def after_accum(st):
                # early per-block tails only for the last group (drains the
                # program tail); elsewhere they would steal ACT slots from
                # the exp stream, which is near-critical mid-schedule
                c = st[0]
                if last and c in kbs:
                    block_tail(kbs.index(c))

            for c in range(nch):
                pend.append(emit_score(c))
                if len(pend) > 2:
                    st = pend.pop(0)
                    emit_accum(st)
                    after_accum(st)
                want = (len(pump) * (2 * (c + 1) - nch)) // max(1, nch) \
                    if 2 * (c + 1) > nch else 0
                while pumped < min(want, len(pump)):
                    pump[pumped]()
                    pumped += 1
            for st in pend:
                emit_accum(st)
                after_accum(st)
            if not last:
                group_tail()
            while pumped < len(pump):
                pump[pumped]()
                pumped += 1

        # ---- orchestration ----
        # tile-0 chains pipeline together right after topk0; topk1's serial
        # DVE rounds queue behind them (emitted after the tile-0 chain DVE
        # steps, overlapping the gA/gB pipelines whose tile-0 blocks carry
        # most of the PE work); tile-1 chains + amt builds are pumped into
        # the gB/gC chunk streams.
        gA, gB, gC, gD = GORDERS[r]
        blocksAB = [4 * gA + j for j in range(4)] + [4 * gB + j for j in range(4)]
        sds_mm(1)
        for t in interleave_chains(blocksAB):
            t()
        thA, amtsA = amt_steps(gA)
        for t in thA:
            t()
        topk_rounds(1)
        thB, amtsB = amt_steps(gB)
        run_group(gA, amtsA, pump=thB)
        chainsC = interleave_chains([4 * gC + j for j in range(4)])
        thC, amtsC = amt_steps(gC)
        run_group(gB, amtsB, pump=chainsC + thC)
        chainsD = interleave_chains([4 * gD + j for j in range(4)])
        thD, amtsD = amt_steps(gD)
        run_group(gC, amtsC, pump=chainsD + thD)
        run_group(gD, amtsD, pump=[], last=True)


_PROGRAMS = {}


def build_program(r: int, debug=False):
    key = (r, debug)
    if key in _PROGRAMS:
        return _PROGRAMS[key]
    nc = bacc.Bacc("TRN2", target_bir_lowering=False, debug=False)
    Qd = nc.dram_tensor("Q", [NLQ, HD], F32R, kind="ExternalInput").ap()
    Kd = nc.dram_tensor("K", [T, HD], F32R, kind="ExternalInput").ap()
    QDSd = nc.dram_tensor("QDS", [NLQ, HD], F32, kind="ExternalInput").ap()
    KDSd = nc.dram_tensor("KDS", [T, HD], F32, kind="ExternalInput").ap()
    Vd = nc.dram_tensor("V", [T, HD], F32R, kind="ExternalInput").ap()
    Ud = nc.dram_tensor("UBT", [128, NB], F32, kind="ExternalInput").ap()
    Od = nc.dram_tensor("OUT", [NLQ, HD], F32, kind="ExternalOutput").ap()
    dbg = None
    if debug:
        dbg = {}
        for nm, shp in (("SDS0", [128, TDS + 1]), ("SDS1", [128, TDS + 1]),
                        ("E0", [128, TDS]), ("F0", [128, TDS]),
                        ("AM0", [128, TDS]), ("PTS0", [128, 512])):
            dbg[nm] = nc.dram_tensor(nm, shp, F32, kind="ExternalOutput").ap()
    with tile.TileContext(nc) as tc:
        _kernel_body(tc, r, Qd, Kd, QDSd, KDSd, Vd, Ud, Od, dbg)
    nc.compile()
    _PROGRAMS[key] = nc
    return nc


def shard_inputs(Q, K, V, U):
    """Per-core input dicts: core = 4*r + h (devices 0-3 parity 0)."""
    maps = []
    Qr = Q[0].reshape(NH, 2 * NB, QPB, HD)
    Ur = U[0].reshape(2 * NB, QPB)
    for r in range(2):
        for h in range(NH):
            qsh = np.ascontiguousarray(Qr[h, KBS[r]].reshape(NLQ, HD))
            ubt = np.ascontiguousarray(Ur[KBS[r]].T)
            ksh = np.ascontiguousarray(K[0, h])
            maps.append({
                "Q": qsh,
                "QDS": qsh,
                "K": ksh,
                "KDS": ksh,
                "V": np.ascontiguousarray(V[0, h]),
                "UBT": ubt,
            })
    return maps


def unshard_output(outs):
    O = np.empty((B, NH, T, HD), np.float32)
    Ov = O[0].reshape(NH, 2 * NB, QPB, HD)
    i = 0
    for r in range(2):
        for h in range(NH):
            Ov[h, KBS[r]] = outs[i]["OUT"].reshape(NB, QPB, HD)
            i += 1
    return O


def _run_concurrent(in_maps):
    """Dispatch parity-0 on devices 0-3 and parity-1 on devices 4-7."""
    import jax
    from jax.sharding import Mesh, PartitionSpec
    from jax.experimental.shard_map import shard_map
    from concourse import bass2jax

    bass2jax.install_neuronx_cc_hook()
    devices = jax.devices()
    assert len(devices) >= 8, f"need 8 neuron cores, got {len(devices)}"

    pending = []
    for r in range(2):
        nc = build_program(r)
        maps = in_maps[4 * r:4 * r + 4]
        pname = nc.partition_id_tensor.name if nc.partition_id_tensor else None
        in_names, out_names, out_avals, zero_outs = [], [], [], []
        for alloc in nc.m.functions[0].allocations:
            if not isinstance(alloc, mybir.MemoryLocationSet):
                continue
            name = alloc.memorylocations[0].name
            if alloc.kind == "ExternalInput":
                if name != pname:
                    in_names.append(name)
            elif alloc.kind == "ExternalOutput":
                out_names.append(name)
                shape = tuple(alloc.tensor_shape)
                dtype = mybir.dt.np(alloc.dtype)
                out_avals.append(jax.core.ShapedArray(shape, dtype))
                zero_outs.append(np.zeros(shape, dtype))
        n_params = len(in_names)
        n_outs = len(out_avals)
        all_names = in_names + out_names
        if pname is not None:
            all_names = all_names + [pname]
        donate = tuple(range(n_params, n_params + n_outs))

        def _body(*args, _nc=nc, _avals=tuple(out_avals),
                  _names=tuple(all_names), _onames=tuple(out_names),
                  _pname=pname):
            operands = list(args)
            if _pname is not None:
                operands.append(bass2jax.partition_id_tensor())
            outs = bass2jax._bass_exec_p.bind(
                *operands,
                out_avals=_avals,
                in_names=_names,
                out_names=_onames,
                lowering_input_output_aliases=(),
                sim_require_finite=True,
                sim_require_nnan=True,
                nc=_nc,
            )
            return tuple(outs)

        mesh = Mesh(np.asarray(devices[4 * r:4 * r + 4]), ("core",))
        in_specs = (PartitionSpec("core"),) * (n_params + n_outs)
        out_specs = (PartitionSpec("core"),) * n_outs
        fn = jax.jit(
            shard_map(_body, mesh=mesh, in_specs=in_specs,
                      out_specs=out_specs, check_rep=False),
            donate_argnums=donate, keep_unused=True,
        )
        per_core = [[np.asarray(m[nm]) for nm in in_names] for m in maps]
        concat_in = [
            np.concatenate([per_core[c][i] for c in range(4)], axis=0)
            for i in range(n_params)
        ]
        concat_zero = [
            np.concatenate([z] * 4, axis=0) for z in zero_outs
        ]
        out_arrs = fn(*concat_in, *concat_zero)
        pending.append((out_arrs, out_names))

    results = []
    for r, (out_arrs, out_names) in enumerate(pending):
        outs = [np.asarray(a) for a in out_arrs]
        for c in range(4):
            d = {}
            for i, nm in enumerate(out_names):
                n0 = outs[i].shape[0] // 4
                d[nm] = outs[i][c * n0:(c + 1) * n0]
            results.append(d)
    return results


def kernel(**inputs):
    Q = np.asarray(inputs["Q"], np.float32)
    K = np.asarray(inputs["K"], np.float32)
    V = np.asarray(inputs["V"], np.float32)
    U = np.asarray(inputs["U"], np.float32)
    in_maps = shard_inputs(Q, K, V, U)
    results = _run_concurrent(in_maps)
    return unshard_output(results)


# revision 7
# speedup vs baseline: 1.4721x; 1.0220x over previous
"""DSALite sparse-attention Trainium2 kernel, transposed-flow redesign.

Problem: B=1, nH=4, T=4096, hd=128 attention where the mask is derived from
8x-downsampled scores: per full row, threshold = 128th largest of the 512
downsampled (and u-scaled) scores, mask = sigmoid((s - thr) * 10 * u) * causal,
scores += (1-mask) * (-1e9), softmax, @V.

Sharding: 8 cores = 4 heads x 2 row-parities (identical to the baseline
kernel).  Core (h, r) handles head h and the 16 query blocks KBS[r].

v2 redesign (vs the q-major baseline): all per-cell work runs in the
TRANSPOSED domain X^T[k, q] so the mask add, the row-max subtraction and the
softmax denominator come out of PE matmuls instead of DVE elementwise ops:

  per 128-k chunk c, per 4-block group (512 q):
    X^T  = K_c Q^T           (fp32r matmul, scale folded into Q^T)
         + Rep(c) @ amin2T   (replicates 16 ds-mask rows onto 128 k rows;
                              amin2 = min(F, ds-causal) + negm pre-folded)
    P^T  = exp(X^T)          (one ACT op PSUM->SBUF(f32r), no bias needed)
    diag chunk: zero k>q half via one gpsimd affine_select (reference
                contributes exactly 0 there: exp(S - 1e9 - M) underflows)
    den += ones^T @ P^T      (PE rank-1 into a [1,512] accumulator)
    O^T += V_c^T @ P^T       (fp32r accumulation, 512 wide)

This deletes the baseline's three big serial DVE/ACT burdens: the X = S+A
elementwise add (51us DVE), the P chunk transposes (34k PE cycles), and the
PSUM->SBUF P^T copies (~20us DVE + ACT).  The mask chain itself (exact f32
sigmoid bucket semantics, top-128 threshold via DVE max8/match_replace) is
carried over op-for-op from the baseline; amin2T is produced by 40 small PE
transposes of the per-block amin2 = amin + negm tiles.

Numerical notes (same bucket-exactness strategy as the baseline):
  - ds scores/threshold/sigmoid/F chain identical to baseline (exact f32).
  - X^T accumulates S~fp32r + amin2 + negm in f32 PSUM adds; only the smooth
    softmax path sees the fp32r rounding, mask buckets are computed exactly.
  - denominator comes from ones@P^T in fp32r (P in [0,1], err ~2^-21 rel).
  - suffix (fully-masked) columns contribute C = exp(-1e9 - M) per row via
    rank-1 SW x C^T injects and a C * n_suffix denominator fixup (baseline
    mechanism, unchanged).
"""

import numpy as np

import concourse.bass as bass
import concourse.bacc as bacc
import concourse.mybir as mybir
import concourse.tile as tile
from concourse.masks import make_identity

F32 = mybir.dt.float32
F32R = mybir.dt.float32r
BF16 = mybir.dt.bfloat16
ALU = mybir.AluOpType
ACTF = mybir.ActivationFunctionType
AX = mybir.AxisListType

B, NH, T, HD = 1, 4, 4096, 128
STRIDE = 8
TDS = T // STRIDE          # 512 downsampled positions
KDS = 128                  # exact 128th largest per ds row
NEG = -1e9
SCALE = HD ** -0.5
ZAP = -1e30

NB = 16                    # 128-row query blocks per core
QPB = 128
NLQ = NB * QPB             # 2048 local query rows
NG = 4                     # groups of 4 blocks (512 q)
CH = T // 128              # 32 key/value chunks
CMW = 1008                 # sliding ds-causal const width
GORDERS = {0: [0, 1, 2, 3], 1: [0, 1, 2, 3]}
# Same per-program block SETS as the baseline (near-equal causal work), but
# ordered so sds-tile 0 (list positions 0-7 = groups A,B) carries the bulk of
# the PE work: group B's big blocks keep PE busy for the ~19us that tile 1's
# serial top-k chain occupies the DVE.  Groups ascend within themselves
# (the jmin narrowing logic requires it); block 0 of the list must be the
# program's earliest block (desperate-row special handling).
KBS = [
    [0, 2, 4, 6, 24, 27, 29, 31, 8, 10, 12, 14, 18, 20, 22, 23],
    [1, 3, 5, 7, 25, 26, 28, 30, 9, 11, 13, 15, 16, 17, 19, 21],
]


def _consts(nc, pool):
    ident = pool.tile([128, 128], F32, tag="ident")
    make_identity(nc, ident[:])
    identr = pool.tile([128, 128], F32R, tag="identr")
    nc.vector.tensor_copy(identr[:], ident[:])

    # cm3[i, jj] = 0.0 if jj <= 496 + i//8 else -1e9 (sliding ds-causal mask)
    cm3 = pool.tile([128, CMW], F32, tag="cm3")
    nc.gpsimd.memset(cm3[:], 0.0)
    nc.gpsimd.affine_select(
        out=cm3[:], in_=cm3[:], pattern=[[-8, CMW]], base=3968,
        channel_multiplier=1, compare_op=ALU.is_ge, fill=NEG,
    )

    # rep[bp][k, i] = 1.0 iff k == 16*bp + i//8 (f32 for the mask chain,
    # f32r copies for the X^T mask-replication matmuls)
    reps, repsr = [], []
    for bp in range(8):
        rep = pool.tile([128, 128], F32, tag=f"rep{bp}")
        nc.gpsimd.memset(rep[:], 1.0)
        nc.gpsimd.affine_select(
            out=rep[:], in_=rep[:], pattern=[[1, 128]], base=128 * bp,
            channel_multiplier=-8, compare_op=ALU.is_ge, fill=0.0)
        nc.gpsimd.affine_select(
            out=rep[:], in_=rep[:], pattern=[[-1, 128]], base=7 - 128 * bp,
            channel_multiplier=8, compare_op=ALU.is_ge, fill=0.0)
        reps.append(rep)
        repr_ = pool.tile([128, 128], F32R, tag=f"repr{bp}")
        nc.vector.tensor_copy(repr_[:], rep[:])
        repsr.append(repr_)

    onesf = pool.tile([128, 1], F32, tag="onesf")
    nc.gpsimd.memset(onesf[:], 1.0)
    onesr = pool.tile([128, 1], F32R, tag="onesr")
    nc.vector.tensor_copy(onesr[:], onesf[:])
    onesb = pool.tile([128, 1], BF16, tag="onesb")
    nc.gpsimd.memset(onesb[:], 1.0)
    onesrow = pool.tile([1, 128], F32, tag="onesrow")
    nc.gpsimd.memset(onesrow[:], 1.0)

    # c01T[i, j] = 1 where i > j (strictly below diagonal in [k, q] layout =
    # above-diagonal in q-major): the region of the special block's diagonal
    # chunk overwritten with the per-row masked constant C.
    c01t = pool.tile([128, 128], mybir.dt.int8, tag="c01t")
    nc.gpsimd.memset(c01t[:], 1)
    nc.gpsimd.affine_select(
        out=c01t[:], in_=c01t[:], pattern=[[-1, 128]], base=-1,
        channel_multiplier=1, compare_op=ALU.is_ge, fill=0,
    )

    return ident, identr, cm3, reps, repsr, onesr, onesb, onesrow, c01t


def _kernel_body(tc, r, Qd, Kd, QDSd, KDSd, Vd, Ud, Od, dbg=None):
    nc = tc.nc
    from contextlib import ExitStack
    with ExitStack() as ctx:
        cpool = ctx.enter_context(tc.tile_pool(name="consts", bufs=1))
        inpool = ctx.enter_context(tc.tile_pool(name="inputs", bufs=1))
        # PSUM budget (8 banks): xps 2 + outt 1 + rsps 1 + maskps 2 + ptps 1
        # + scratch 1
        xpool = ctx.enter_context(tc.tile_pool(name="xps", bufs=2, space="PSUM"))
        outtpool = ctx.enter_context(tc.tile_pool(name="outtps", bufs=1, space="PSUM"))
        rspool = ctx.enter_context(tc.tile_pool(name="rsps", bufs=1, space="PSUM"))
        mpspool = ctx.enter_context(tc.tile_pool(name="maskps", bufs=2, space="PSUM"))
        ptpool = ctx.enter_context(tc.tile_pool(name="ptps", bufs=1, space="PSUM"))
        scrpspool = ctx.enter_context(tc.tile_pool(name="scrps", bufs=1, space="PSUM"))
        sdspool = ctx.enter_context(tc.tile_pool(name="sds", bufs=1))
        scrpool = ctx.enter_context(tc.tile_pool(name="scr", bufs=1))
        epool = ctx.enter_context(tc.tile_pool(name="e", bufs=4))
        fpool = ctx.enter_context(tc.tile_pool(name="f", bufs=5))
        aminpool = ctx.enter_context(tc.tile_pool(name="amin", bufs=3))
        am2pool = ctx.enter_context(tc.tile_pool(name="am2", bufs=8))
        amtpool = ctx.enter_context(tc.tile_pool(name="amt", bufs=7))
        smallpool = ctx.enter_context(tc.tile_pool(name="small", bufs=NB))
        tinypool = ctx.enter_context(tc.tile_pool(name="tiny", bufs=6))
        ptspool = ctx.enter_context(tc.tile_pool(name="pts", bufs=4))
        outtsbpool = ctx.enter_context(tc.tile_pool(name="outtsb", bufs=3))
        swsbpool = ctx.enter_context(tc.tile_pool(name="swsb", bufs=1))

        (ident, identr, cm3, reps, repsr, onesr, onesb, onesrow,
         c01t) = _consts(nc, cpool)

        # one shared PSUM scratch bank; disjoint regions, subtile-dep tracked
        scrps = scrpspool.tile([128, 512], F32, tag="scrps")
        swp = scrps[0:1, 0:128]        # suffix colsum accumulator
        ps1b_r = scrps[0:128, 128:129]  # threshold replicate matmul out
        ctp_r = scrps[0:1, 192:320]     # cee^T transpose out
        rst_r = scrps[0:128, 352:353]   # rowsum^T transpose out
        ngt_r = scrps[0:1, 384:512]     # negm^T transpose out (block 0)

        # ---- loads: ds subsets first (they gate the serial top-k chain) ----
        kt = inpool.tile([128, T], F32R, tag="kt")     # K^T [d, t]
        qts = inpool.tile([128, NLQ], F32R, tag="qts")  # Q^T * scale [d, q]
        kdst = inpool.tile([128, TDS], F32, tag="kdst")
        qdst = inpool.tile([128, TDS // 2], F32, tag="qdst")
        prep = ctx.enter_context(tc.tile_pool(name="prep", bufs=2))
        for dsrc, dsdst, nds in ((KDSd, kdst, TDS), (QDSd, qdst, TDS // 2)):
            natd = prep.tile([128, nds], F32, tag="natd", name="natd")
            nc.sync.dma_start(
                out=natd[:].rearrange("p (c d) -> p c d", d=128),
                in_=dsrc.rearrange("(c p s) d -> p c s d", p=128,
                                   s=STRIDE)[:, :, 0, :],
            )
            n3 = natd[:].rearrange("p (c d) -> p c d", d=128)
            ptd = ptpool.tile([128, 512], F32, tag="ptps", name="ppsd")
            for j in range(nds // 128):
                nc.tensor.transpose(
                    ptd[:, 128 * j:128 * j + 128], n3[:, j, :], ident[:])
            # ACT copy: keeps the DVE queue free so topk0 starts immediately
            nc.scalar.copy(dsdst[:], ptd[:, 0:nds])

        ub = inpool.tile([128, NB], F32, tag="ub")
        nc.sync.dma_start(out=ub[:], in_=Ud[:])
        usc = inpool.tile([128, NB], F32, tag="usc")
        nc.vector.tensor_scalar(usc[:], ub[:], 0.0, 1.0, op0=ALU.max,
                                op1=ALU.min)
        nc.vector.tensor_scalar(usc[:], usc[:], 1.0, None, op0=ALU.add)
        vsb = inpool.tile([128, T], F32R, tag="vsb")   # [t, d] natural

        def v_load():
            # V early: the suffix colsums are the main PE work available
            # under the serial topk0 DVE chain
            nc.sync.dma_start(
                out=vsb[:].rearrange("p (c d) -> p c d", d=128),
                in_=Vd.rearrange("(c p) d -> p c d", p=128),
            )

        def full_prep():
            # K^T: 8 pieces of 512 cols; f32r transposes (1.5 cyc/row)
            s3 = Kd.rearrange("(c p) d -> p c d", p=128)
            for c4 in range(8):
                nat = prep.tile([128, 512], F32R, tag="nat")
                nc.sync.dma_start(
                    out=nat[:].rearrange("p (c d) -> p c d", d=128),
                    in_=s3[:, 4 * c4:4 * c4 + 4, :],
                )
                n3 = nat[:].rearrange("p (c d) -> p c d", d=128)
                pt = ptpool.tile([128, 512], F32R, tag="ptps")
                for j in range(4):
                    nc.tensor.transpose(
                        pt[:, 128 * j:128 * j + 128], n3[:, j, :], identr[:])
                nc.any.tensor_copy(kt[:, 512 * c4:512 * c4 + 512], pt[:])
            # Q^T with the score scale folded into the PSUM->SBUF copy
            s3 = Qd.rearrange("(c p) d -> p c d", p=128)
            for c4 in range(4):
                nat = prep.tile([128, 512], F32R, tag="nat")
                nc.sync.dma_start(
                    out=nat[:].rearrange("p (c d) -> p c d", d=128),
                    in_=s3[:, 4 * c4:4 * c4 + 4, :],
                )
                n3 = nat[:].rearrange("p (c d) -> p c d", d=128)
                pt = ptpool.tile([128, 512], F32R, tag="ptps")
                for j in range(4):
                    nc.tensor.transpose(
                        pt[:, 128 * j:128 * j + 128], n3[:, j, :], identr[:])
                nc.scalar.mul(qts[:, 512 * c4:512 * c4 + 512], pt[:], SCALE)

        # ---- downsampled scores + exact per-row 128th largest ----
        # Split so sds matmuls (PE) can be emitted early while the serial
        # DVE top-k rounds are placed where the DVE queue wants them.
        sds_tiles = {}

        def sds_mm(t):
            sds = sdspool.tile([128, TDS + 1], F32, tag=f"sds{t}",
                               name=f"sds{t}")
            sds_tiles[t] = sds
            ps = mpspool.tile([128, 512], F32, tag="maskps", name=f"sdsps{t}")
            nc.tensor.matmul(ps[:], qdst[:, 128 * t:128 * t + 128], kdst[:])
            nc.scalar.mul(sds[:, 0:TDS], ps[:], SCALE)

        def topk_rounds(t):
            sds = sds_tiles[t]
            scr = scrpool.tile([128, TDS], F32, tag="scr", name=f"scr{t}")
            nc.vector.tensor_copy(scr[:], sds[:, 0:TDS])
            maxsc = scrpool.tile([128, 8], F32, tag="maxsc", name=f"maxsc{t}")
            for rnd in range(KDS // 8):
                nc.vector.max(out=maxsc[:], in_=scr[:])
                if rnd < KDS // 8 - 1:
                    nc.vector.match_replace(
                        out=scr[:], in_to_replace=maxsc[:], in_values=scr[:],
                        imm_value=ZAP,
                    )
            nc.vector.tensor_copy(sds[:, TDS:TDS + 1], maxsc[:, 7:8])
            if dbg is not None:
                nc.sync.dma_start(out=dbg[f"SDS{t}"], in_=sds[:])

        sds_mm(0)
        topk_rounds(0)
        v_load()
        full_prep()

        # ---- suffix V column-sums SW(cb), [1, 128] slices in swall ----
        # bf16 V copy makes the narrow [1,128]-out colsum matmuls 1 cyc/row
        # (fp32r pays a 4x penalty below 256-wide); SW only feeds desperate
        # rows via C = exp(-1e9 - M), so 0.4% bf16 rounding is irrelevant.
        osb = inpool.tile([128, NB * 128], F32, tag="osb")
        vsbb = inpool.tile([128, T], BF16, tag="vsbb")
        nc.gpsimd.tensor_copy(vsbb[:], vsb[:])
        swall = swsbpool.tile([1, NB * 128], BF16, tag="swall")
        call = swsbpool.tile([1, NB * 128], BF16, tag="call")
        negmrow4 = swsbpool.tile([1, 512], F32, tag="negmrow4")
        ceerow0 = swsbpool.tile([1, 128], F32, tag="ceerow0")
        amtf = inpool.tile([128, 512], F32, tag="amtf")   # gA amin^T (f32)
        # cbm[k, q] = C(q) for k > q else 0: the diag chunk's above-diagonal
        # dust of the desperate-capable block, injected via two rank-128
        # matmuls into rsps and outt (pts itself stays affine-zeroed there)
        cbm = inpool.tile([128, 128], F32R, tag="cbm")
        emitted = 0
        prev = CH
        for b in sorted(range(NB), key=lambda b_: KBS[r][b_], reverse=True):
            cb = KBS[r][b] + 1
            for c in range(cb, prev):
                emitted += 1
                nc.tensor.matmul(swp, onesb[:], vsbb[:, 128 * c:128 * c + 128],
                                 start=(emitted == 1), stop=(c == cb),
                                 skip_group_check=(emitted != 1))
            prev = cb
            if cb <= CH - 1:
                nc.scalar.copy(swall[0:1, 128 * b:128 * b + 128], swp)

        # ---- mask chain (q-major, bucket-exact; -> amin2 f32r + cee) ----
        # Emitted as per-block STEP thunks so 8 blocks' chains interleave
        # (pipelining the 12-step cross-engine latency chain) and so steps can
        # be pumped into the chunk pipelines of earlier groups.
        negms, cees, am2s = {}, {}, {}

        def chain_steps(b):
            kb = KBS[r][b]
            nd = 16 * (kb + 1)
            tt, pp = divmod(b, 8)
            st = {}

            def s0():
                sds = sds_tiles[tt]
                st["ps"] = mpspool.tile([128, 512], F32, tag="maskps", name="chps")
                nc.tensor.matmul(st["ps"][:, 0:nd], reps[pp][:], sds[:, 0:nd])
                nc.tensor.matmul(ps1b_r, reps[pp][:], sds[:, TDS:TDS + 1])
                st["thru"] = tinypool.tile([128, 1], F32, tag="thru", name="thru")
                nc.vector.tensor_scalar(st["thru"][:], ps1b_r, usc[:, b:b + 1],
                                        None, op0=ALU.mult)

            def s1():
                # m = 1/(1 + exp(-10*arg)); bucket-exact vs XLA f32 sigmoid
                st["arg"] = epool.tile([128, TDS], F32, tag="arg", name="arg")
                nc.vector.tensor_scalar(st["arg"][:, 0:nd], st["ps"][:, 0:nd],
                                        usc[:, b:b + 1], st["thru"][:],
                                        op0=ALU.mult, op1=ALU.subtract)

            def s2():
                st["z"] = epool.tile([128, TDS], F32, tag="z", name="z")
                nc.scalar.activation(st["z"][:, 0:nd], st["arg"][:, 0:nd],
                                     ACTF.Exp, scale=-10.0)

            def s3():
                nc.vector.tensor_scalar(st["z"][:, 0:nd], st["z"][:, 0:nd],
                                        1.0, None, op0=ALU.add)

            def s4():
                st["e"] = epool.tile([128, TDS], F32, tag="e", name="e")
                nc.vector.reciprocal(st["e"][:, 0:nd], st["z"][:, 0:nd])

            def s5():
                st["f"] = fpool.tile([128, TDS], F32, tag="f", name="f")
                nc.gpsimd.tensor_scalar(st["f"][:, 0:nd], st["e"][:, 0:nd],
                                        -1.0, 1.0, op0=ALU.mult, op1=ALU.add)

            def s6():
                nc.gpsimd.tensor_scalar(st["f"][:, 0:nd], st["f"][:, 0:nd],
                                        NEG, None, op0=ALU.mult)

            def s7():
                # amin = min(F, ds-causal): F <= 0 always and cm3 is 0 on all
                # fully-allowed columns, so the min only matters on the last
                # 16 ds cols (the diagonal ds band, cm3[:, 496:512]) -- apply
                # it in place on f, 16 cols instead of nd.
                nc.vector.tensor_tensor(st["f"][:, nd - 16:nd],
                                        st["f"][:, nd - 16:nd],
                                        cm3[:, 496:512], op=ALU.min)
                st["amin"] = st["f"]

            def s8():
                negm = smallpool.tile([128, 1], F32, tag="negm")
                nc.vector.tensor_reduce(negm[:], st["amin"][:, 0:nd], axis=AX.X,
                                        op=ALU.max, negate=True)
                negms[b] = negm

            def s9():
                if b < 4:
                    # group A blocks can hold desperate or SEMI-desperate
                    # rows (|max F| large): the reference's fl(S + F)
                    # quantizes S at ulp(F), so F must stay UNFOLDED in
                    # exact f32 and negm must be added only after the S+F
                    # rounding (fl(fl(S+F) + negm), matching the ref)
                    amu = am2pool.tile([128, TDS], F32, tag="am2", name="amu")
                    nc.vector.tensor_copy(amu[:, 0:nd], st["amin"][:, 0:nd])
                    am2s[b] = amu
                    return
                # amin2 = amin + negm (X^T = S + amin2 needs no later bias);
                # safe here: every masked column of these blocks has C == 0
                am2 = am2pool.tile([128, TDS], F32R, tag="am2", name="am2")
                nc.gpsimd.tensor_scalar(am2[:, 0:nd], st["amin"][:, 0:nd],
                                        negms[b][:], None, op0=ALU.add)
                am2s[b] = am2

            def s10():
                b2 = tinypool.tile([128, 1], F32, tag="b2")
                nc.vector.tensor_scalar(b2[:], negms[b][:], NEG, None,
                                        op0=ALU.add)
                cee = smallpool.tile([128, 1], F32, tag="cee")
                nc.scalar.activation(cee[:], b2[:], ACTF.Exp)
                cees[b] = cee

            def s11():
                kb_ = KBS[r][b]
                if kb_ + 1 <= CH - 1:
                    nc.tensor.transpose(ctp_r, cees[b][:], ident[:])
                    nc.scalar.copy(call[0:1, 128 * b:128 * b + 128], ctp_r)
                if b < 4:
                    # f32 rows for the exact rank-1 negm inject (+ C bcast)
                    if b == 0:
                        nc.scalar.copy(ceerow0[:], ctp_r)
                    nc.tensor.transpose(ngt_r, negms[b][:], ident[:])
                    nc.scalar.copy(negmrow4[0:1, 128 * b:128 * b + 128],
                                   ngt_r)
                if dbg is not None and b == 0:
                    nc.sync.dma_start(out=dbg["E0"], in_=st["e"][:])
                    nc.sync.dma_start(out=dbg["AM0"], in_=st["amin"][:])

            return [s0, s1, s2, s3, s4, s5, s6, s7, s8, s9, s10, s11]

        def interleave_chains(blocks):
            """Wavefront-interleaved chain thunks: chain i runs one step
            behind chain i-1, so pool rotation stays alloc-after-consumer
            and every wave packs different engines."""
            chains = [chain_steps(b) for b in blocks]
            out = []
            for w in range(len(chains) + 11):
                for i, ch in enumerate(chains):
                    s = w - i
                    if 0 <= s < 12:
                        out.append(ch[s])
            return out

        def amt_steps(g):
            """amin2T piece-build thunks for group g; returns (thunks, amts)."""
            kbs = [KBS[r][4 * g + j] for j in range(4)]
            cmax = kbs[3]
            npieces = (cmax + 1 + 7) // 8
            amts = []
            thunks = []
            if g == 0:
                # group A: one exact-f32 amin^T tile for all four blocks
                # (no folded pieces) + block 0's masked-C broadcast tile
                def build_exact(j):
                    def run():
                        b = j
                        nd = 16 * (kbs[j] + 1)
                        pt = ptpool.tile([128, 512], F32, tag="ptps",
                                         name="ptspec")
                        nc.tensor.transpose(pt[0:nd, 128 * j:128 * j + 128],
                                            am2s[b][:, 0:nd], ident[:])
                        nc.any.tensor_copy(
                            amtf[0:nd, 128 * j:128 * j + 128],
                            pt[0:nd, 128 * j:128 * j + 128])
                        if j == 0:
                            nc.tensor.matmul(pt[:, 128:256], onesrow[:],
                                             ceerow0[:])
                            nc.vector.tensor_copy(cbm[:], pt[:, 128:256])
                            nc.gpsimd.affine_select(
                                out=cbm[:], in_=cbm[:], pattern=[[-1, 128]],
                                base=-1, channel_multiplier=1,
                                compare_op=ALU.is_ge, fill=0.0)
                    return run
                for j in range(4):
                    thunks.append(build_exact(j))
                return thunks, amts
            for p in range(npieces):
                amt = amtpool.tile([128, 512], F32R, tag="amt",
                                   name=f"amt{g}_{p}")
                amts.append(amt)

                def build(p=p, amt=amt):
                    pt = ptpool.tile([128, 512], F32R, tag="ptps")
                    for j in range(4):
                        b = 4 * g + j
                        nd = 16 * (kbs[j] + 1)
                        w = min(128, nd - 128 * p)
                        if w <= 0:
                            continue
                        nc.tensor.transpose(
                            pt[0:w, 128 * j:128 * j + 128],
                            am2s[b][:, 128 * p:128 * p + w], identr[:])
                        nc.any.tensor_copy(amt[0:w, 128 * j:128 * j + 128],
                                           pt[0:w, 128 * j:128 * j + 128])
                thunks.append(build)
            return thunks, amts

        # ---- group chunk pipeline ----
        def run_group(g, amts, pump, last=False):
            kbs = [KBS[r][4 * g + j] for j in range(4)]
            cmax = kbs[3]

            outt = outtpool.tile([128, 512], F32, tag="outtps")
            rsps = rspool.tile([1, 512], F32, tag="rsps")

            def emit_score(c):
                jmin = min(j for j in range(4) if kbs[j] >= c)
                lo = 128 * jmin
                klo = 16 * (c % 8)
                xps = xpool.tile([128, 512], F32, tag="xps")
                nc.tensor.matmul(xps[:, lo:512], kt[:, 128 * c:128 * c + 128],
                                 qts[:, 512 * g + lo:512 * g + 512],
                                 start=True, stop=False)
                if g == 0:
                    # group A: exact-f32 A-rep then exact rank-1 negm, so
                    # fl(fl(S + F) + negm) matches the reference bit-level
                    # quantization (F magnitudes up to 1e9 here quantize S)
                    nc.tensor.matmul(xps[:, lo:512],
                                     reps[c % 8][0:klo + 16, :],
                                     amtf[0:klo + 16, lo:512],
                                     start=False, stop=False,
                                     skip_group_check=True)
                    nc.tensor.matmul(xps[:, lo:512], onesrow[:],
                                     negmrow4[0:1, lo:512],
                                     start=False, stop=True,
                                     skip_group_check=True)
                else:
                    nc.tensor.matmul(xps[:, lo:512],
                                     repsr[c % 8][0:klo + 16, :],
                                     amts[c // 8][0:klo + 16, lo:512],
                                     start=False, stop=True,
                                     skip_group_check=True)
                pts = ptspool.tile([128, 512], F32R, tag="pts")
                nc.scalar.activation(pts[:, lo:512], xps[:, lo:512], ACTF.Exp)
                if c in kbs:
                    j = kbs.index(c)
                    # zero strictly-above-diagonal (k > q); for the special
                    # block the C dust there is re-injected via cbm matmuls
                    nc.gpsimd.affine_select(
                        out=pts[:, 128 * j:128 * j + 128],
                        in_=pts[:, 128 * j:128 * j + 128],
                        pattern=[[1, 128]], base=0, channel_multiplier=-1,
                        compare_op=ALU.is_ge, fill=0.0)
                if dbg is not None and g == 0 and c == 0:
                    nc.sync.dma_start(out=dbg["PTS0"], in_=pts[:])
                return c, lo, pts

            def emit_accum(st):
                # stop whenever some block's columns see their last chunk, so
                # that block's tail (injects/den/epilogue) can read its psum
                # region while the rest keeps accumulating (swp pattern)
                c, lo, pts = st
                stops = (c == cmax) or (last and c in kbs)
                nc.tensor.matmul(rsps[0:1, lo:512], onesr[:], pts[:, lo:512],
                                 start=(c == 0), stop=stops,
                                 skip_group_check=(c != 0))
                nc.tensor.matmul(outt[:, lo:512],
                                 vsb[:, 128 * c:128 * c + 128],
                                 pts[:, lo:512],
                                 start=(c == 0), stop=stops,
                                 skip_group_check=(c != 0))

            def block_tail(j):
                """Emit block j's suffix injects, denominator, O^T->O
                transpose, 1/den scale, and output DMA; valid as soon as
                chunk kbs[j]'s accumulation has stopped."""
                b = 4 * g + j
                nsuf = T - 128 * (kbs[j] + 1)
                if kbs[j] + 1 <= CH - 1:
                    nc.tensor.matmul(
                        outt[:, 128 * j:128 * j + 128],
                        swall[0:1, 128 * b:128 * b + 128],
                        call[0:1, 128 * b:128 * b + 128],
                        start=False, stop=True, skip_group_check=True,
                    )
                if g == 0 and j == 0:
                    # block 0's above-diagonal C dust (desperate rows)
                    dc = kbs[0]
                    nc.tensor.matmul(rsps[0:1, 0:128], onesr[:], cbm[:],
                                     start=False, stop=True,
                                     skip_group_check=True)
                    nc.tensor.matmul(outt[:, 0:128],
                                     vsb[:, 128 * dc:128 * dc + 128], cbm[:],
                                     start=False, stop=True,
                                     skip_group_check=True)
                rs_sb = tinypool.tile([1, 128], F32, tag="rssb")
                nc.scalar.copy(rs_sb[:], rsps[0:1, 128 * j:128 * j + 128])
                nc.tensor.transpose(rst_r, rs_sb[:], ident[0:1, 0:1])
                den = tinypool.tile([128, 1], F32, tag="den")
                if nsuf > 0:
                    nc.vector.scalar_tensor_tensor(
                        out=den[:], in0=cees[b][:], scalar=float(nsuf),
                        in1=rst_r, op0=ALU.mult, op1=ALU.add)
                else:
                    nc.vector.tensor_copy(den[:], rst_r)
                rsum = smallpool.tile([128, 1], F32, tag="rsum")
                nc.vector.reciprocal(rsum[:], den[:])
                outt_sb = outtsbpool.tile([128, 128], F32, tag="outtsb")
                nc.scalar.copy(outt_sb[:], outt[:, 128 * j:128 * j + 128])
                ops = ptpool.tile([128, 512], F32, tag="ptps")
                nc.tensor.transpose(ops[:, 0:128], outt_sb[:], ident[:])
                nc.vector.tensor_scalar(osb[:, 128 * b:128 * b + 128],
                                        ops[:, 0:128], rsum[:], None,
                                        op0=ALU.mult)
                row0 = 128 * b
                nc.sync.dma_start(out=Od[row0:row0 + 128, :],
                                  in_=osb[:, 128 * b:128 * b + 128])

            def group_tail():
                """Whole-group epilogue (non-last groups): fewer, wider ACT
                ops than four block tails."""
                for j in range(4):
                    b = 4 * g + j
                    if kbs[j] + 1 <= CH - 1:
                        nc.tensor.matmul(
                            outt[:, 128 * j:128 * j + 128],
                            swall[0:1, 128 * b:128 * b + 128],
                            call[0:1, 128 * b:128 * b + 128],
                            start=False, stop=True, skip_group_check=True,
                        )
                if g == 0:
                    dc = kbs[0]
                    nc.tensor.matmul(rsps[0:1, 0:128], onesr[:], cbm[:],
                                     start=False, stop=True,
                                     skip_group_check=True)
                    nc.tensor.matmul(outt[:, 0:128],
                                     vsb[:, 128 * dc:128 * dc + 128], cbm[:],
                                     start=False, stop=True,
                                     skip_group_check=True)
                rs_sb = tinypool.tile([1, 512], F32, tag="rssbw")
                nc.scalar.copy(rs_sb[:], rsps[:])
                rsums = []
                for j in range(4):
                    b = 4 * g + j
                    nsuf = T - 128 * (kbs[j] + 1)
                    nc.tensor.transpose(rst_r, rs_sb[0:1, 128 * j:128 * j + 128],
                                        ident[0:1, 0:1])
                    den = tinypool.tile([128, 1], F32, tag="den")
                    if nsuf > 0:
                        nc.vector.scalar_tensor_tensor(
                            out=den[:], in0=cees[b][:], scalar=float(nsuf),
                            in1=rst_r, op0=ALU.mult, op1=ALU.add)
                    else:
                        nc.vector.tensor_copy(den[:], rst_r)
                    rsum = smallpool.tile([128, 1], F32, tag="rsum")
                    nc.vector.reciprocal(rsum[:], den[:])
                    rsums.append(rsum)
                outt_sb = outtsbpool.tile([128, 512], F32, tag="outtsbw")
                nc.scalar.copy(outt_sb[:], outt[:])
                ops = ptpool.tile([128, 512], F32, tag="ptps")
                for j in range(4):
                    nc.tensor.transpose(
                        ops[:, 128 * j:128 * j + 128],
                        outt_sb[:, 128 * j:128 * j + 128], ident[:])
                for j in range(4):
                    b = 4 * g + j
                    nc.vector.tensor_scalar(osb[:, 128 * b:128 * b + 128],
                                            ops[:, 128 * j:128 * j + 128],
                                            rsums[j][:], None, op0=ALU.mult)
                    row0 = 128 * b
                    nc.sync.dma_start(out=Od[row0:row0 + 128, :],
                                      in_=osb[:, 128 * b:128 * b + 128])

            # software-pipelined emission, 2 chunks deep: S/A/exp of chunk c
            # go ahead of rowsum/PV of chunk c-2, so the in-order PE queue
            # has two chunks of slack against ACT exp jitter.  Block tails
            # are emitted as soon as their accumulation stops.  `pump` thunks
            # (later groups' mask chains / amt builds) are spread through the
            # back half of the chunk stream.
            nch = cmax + 1
            pumped = 0
            pend = []

            def after_accum(st):
                # early per-block tails only for the last group (drains the
                # program tail); elsewhere they would steal ACT slots from
                # the exp stream, which is near-critical mid-schedule
                c = st[0]
                if last and c in kbs:
                    block_tail(kbs.index(c))

            for c in range(nch):
                pend.append(emit_score(c))
                if len(pend) > 2:
                    st = pend.pop(0)
                    emit_accum(st)
                    after_accum(st)
                want = (len(pump) * (2 * (c + 1) - nch)) // max(1, nch) \
                    if 2 * (c + 1) > nch else 0
                while pumped < min(want, len(pump)):
                    pump[pumped]()
                    pumped += 1
            for st in pend:
                emit_accum(st)
                after_accum(st)
            if not last:
                group_tail()
            while pumped < len(pump):
                pump[pumped]()
                pumped += 1

        # ---- orchestration ----
        # tile-0 chains pipeline together right after topk0; topk1's serial
        # DVE rounds queue behind them (emitted after the tile-0 chain DVE
        # steps, overlapping the gA/gB pipelines whose tile-0 blocks carry
        # most of the PE work); tile-1 chains + amt builds are pumped into
        # the gB/gC chunk streams.
        gA, gB, gC, gD = GORDERS[r]
        blocksAB = [4 * gA + j for j in range(4)] + [4 * gB + j for j in range(4)]
        sds_mm(1)
        for t in interleave_chains(blocksAB):
            t()
        thA, amtsA = amt_steps(gA)
        for t in thA:
            t()
        topk_rounds(1)
        thB, amtsB = amt_steps(gB)
        run_group(gA, amtsA, pump=thB)
        chainsC = interleave_chains([4 * gC + j for j in range(4)])
        thC, amtsC = amt_steps(gC)
        run_group(gB, amtsB, pump=chainsC + thC)
        chainsD = interleave_chains([4 * gD + j for j in range(4)])
        thD, amtsD = amt_steps(gD)
        run_group(gC, amtsC, pump=chainsD + thD)
        run_group(gD, amtsD, pump=[], last=True)


_PROGRAMS = {}


def build_program(r: int, debug=False):
    key = (r, debug)
    if key in _PROGRAMS:
        return _PROGRAMS[key]
    nc = bacc.Bacc("TRN2", target_bir_lowering=False, debug=False)
    Qd = nc.dram_tensor("Q", [NLQ, HD], F32R, kind="ExternalInput").ap()
    Kd = nc.dram_tensor("K", [T, HD], F32R, kind="ExternalInput").ap()
    QDSd = nc.dram_tensor("QDS", [NLQ, HD], F32, kind="ExternalInput").ap()
    KDSd = nc.dram_tensor("KDS", [T, HD], F32, kind="ExternalInput").ap()
    Vd = nc.dram_tensor("V", [T, HD], F32R, kind="ExternalInput").ap()
    Ud = nc.dram_tensor("UBT", [128, NB], F32, kind="ExternalInput").ap()
    Od = nc.dram_tensor("OUT", [NLQ, HD], F32, kind="ExternalOutput").ap()
    dbg = None
    if debug:
        dbg = {}
        for nm, shp in (("SDS0", [128, TDS + 1]), ("SDS1", [128, TDS + 1]),
                        ("E0", [128, TDS]), ("F0", [128, TDS]),
                        ("AM0", [128, TDS]), ("PTS0", [128, 512])):
            dbg[nm] = nc.dram_tensor(nm, shp, F32, kind="ExternalOutput").ap()
    with tile.TileContext(nc) as tc:
        _kernel_body(tc, r, Qd, Kd, QDSd, KDSd, Vd, Ud, Od, dbg)
    nc.compile()
    _PROGRAMS[key] = nc
    return nc


def shard_inputs(Q, K, V, U):
    """Per-core input dicts: core = 4*r + h (devices 0-3 parity 0)."""
    maps = []
    Qr = Q[0].reshape(NH, 2 * NB, QPB, HD)
    Ur = U[0].reshape(2 * NB, QPB)
    for r in range(2):
        for h in range(NH):
            qsh = np.ascontiguousarray(Qr[h, KBS[r]].reshape(NLQ, HD))
            ubt = np.ascontiguousarray(Ur[KBS[r]].T)
            ksh = np.ascontiguousarray(K[0, h])
            maps.append({
                "Q": qsh,
                "QDS": qsh,
                "K": ksh,
                "KDS": ksh,
                "V": np.ascontiguousarray(V[0, h]),
                "UBT": ubt,
            })
    return maps


def unshard_output(outs):
    O = np.empty((B, NH, T, HD), np.float32)
    Ov = O[0].reshape(NH, 2 * NB, QPB, HD)
    i = 0
    for r in range(2):
        for h in range(NH):
            Ov[h, KBS[r]] = outs[i]["OUT"].reshape(NB, QPB, HD)
            i += 1
    return O


def _run_concurrent(in_maps):
    """Dispatch parity-0 on devices 0-3 and parity-1 on devices 4-7."""
    import jax
    from jax.sharding import Mesh, PartitionSpec
    from jax.experimental.shard_map import shard_map
    from concourse import bass2jax

    bass2jax.install_neuronx_cc_hook()
    devices = jax.devices()
    assert len(devices) >= 8, f"need 8 neuron cores, got {len(devices)}"

    pending = []
    for r in range(2):
        nc = build_program(r)
        maps = in_maps[4 * r:4 * r + 4]
        pname = nc.partition_id_tensor.name if nc.partition_id_tensor else None
        in_names, out_names, out_avals, zero_outs = [], [], [], []
        for alloc in nc.m.functions[0].allocations:
            if not isinstance(alloc, mybir.MemoryLocationSet):
                continue
            name = alloc.memorylocations[0].name
            if alloc.kind == "ExternalInput":
                if name != pname:
                    in_names.append(name)
            elif alloc.kind == "ExternalOutput":
                out_names.append(name)
                shape = tuple(alloc.tensor_shape)
                dtype = mybir.dt.np(alloc.dtype)
                out_avals.append(jax.core.ShapedArray(shape, dtype))
                zero_outs.append(np.zeros(shape, dtype))
        n_params = len(in_names)
        n_outs = len(out_avals)
        all_names = in_names + out_names
        if pname is not None:
            all_names = all_names + [pname]
        donate = tuple(range(n_params, n_params + n_outs))

        def _body(*args, _nc=nc, _avals=tuple(out_avals),
                  _names=tuple(all_names), _onames=tuple(out_names),
                  _pname=pname):
            operands = list(args)
            if _pname is not None:
                operands.append(bass2jax.partition_id_tensor())
            outs = bass2jax._bass_exec_p.bind(
                *operands,
                out_avals=_avals,
                in_names=_names,
                out_names=_onames,
                lowering_input_output_aliases=(),
                sim_require_finite=True,
                sim_require_nnan=True,
                nc=_nc,
            )
            return tuple(outs)

        mesh = Mesh(np.asarray(devices[4 * r:4 * r + 4]), ("core",))
        in_specs = (PartitionSpec("core"),) * (n_params + n_outs)
        out_specs = (PartitionSpec("core"),) * n_outs
        fn = jax.jit(
            shard_map(_body, mesh=mesh, in_specs=in_specs,
                      out_specs=out_specs, check_rep=False),
            donate_argnums=donate, keep_unused=True,
        )
        per_core = [[np.asarray(m[nm]) for nm in in_names] for m in maps]
        concat_in = [
            np.concatenate([per_core[c][i] for c in range(4)], axis=0)
            for i in range(n_params)
        ]
        concat_zero = [
            np.concatenate([z] * 4, axis=0) for z in zero_outs
        ]
        out_arrs = fn(*concat_in, *concat_zero)
        pending.append((out_arrs, out_names))

    results = []
    for r, (out_arrs, out_names) in enumerate(pending):
        outs = [np.asarray(a) for a in out_arrs]
        for c in range(4):
            d = {}
            for i, nm in enumerate(out_names):
                n0 = outs[i].shape[0] // 4
                d[nm] = outs[i][c * n0:(c + 1) * n0]
            results.append(d)
    return results


def kernel(**inputs):
    Q = np.asarray(inputs["Q"], np.float32)
    K = np.asarray(inputs["K"], np.float32)
    V = np.asarray(inputs["V"], np.float32)
    U = np.asarray(inputs["U"], np.float32)
    in_maps = shard_inputs(Q, K, V, U)
    results = _run_concurrent(in_maps)
    return unshard_output(results)


# revision 8
# speedup vs baseline: 1.4727x; 1.0004x over previous
"""DSALite sparse-attention Trainium2 kernel, transposed-flow redesign.

Problem: B=1, nH=4, T=4096, hd=128 attention where the mask is derived from
8x-downsampled scores: per full row, threshold = 128th largest of the 512
downsampled (and u-scaled) scores, mask = sigmoid((s - thr) * 10 * u) * causal,
scores += (1-mask) * (-1e9), softmax, @V.

Sharding: 8 cores = 4 heads x 2 row-parities (identical to the baseline
kernel).  Core (h, r) handles head h and the 16 query blocks KBS[r].

v2 redesign (vs the q-major baseline): all per-cell work runs in the
TRANSPOSED domain X^T[k, q] so the mask add, the row-max subtraction and the
softmax denominator come out of PE matmuls instead of DVE elementwise ops:

  per 128-k chunk c, per 4-block group (512 q):
    X^T  = K_c Q^T           (fp32r matmul, scale folded into Q^T)
         + Rep(c) @ amin2T   (replicates 16 ds-mask rows onto 128 k rows;
                              amin2 = min(F, ds-causal) + negm pre-folded)
    P^T  = exp(X^T)          (one ACT op PSUM->SBUF(f32r), no bias needed)
    diag chunk: zero k>q half via one gpsimd affine_select (reference
                contributes exactly 0 there: exp(S - 1e9 - M) underflows)
    den += ones^T @ P^T      (PE rank-1 into a [1,512] accumulator)
    O^T += V_c^T @ P^T       (fp32r accumulation, 512 wide)

This deletes the baseline's three big serial DVE/ACT burdens: the X = S+A
elementwise add (51us DVE), the P chunk transposes (34k PE cycles), and the
PSUM->SBUF P^T copies (~20us DVE + ACT).  The mask chain itself (exact f32
sigmoid bucket semantics, top-128 threshold via DVE max8/match_replace) is
carried over op-for-op from the baseline; amin2T is produced by 40 small PE
transposes of the per-block amin2 = amin + negm tiles.

Numerical notes (same bucket-exactness strategy as the baseline):
  - ds scores/threshold/sigmoid/F chain identical to baseline (exact f32).
  - X^T accumulates S~fp32r + amin2 + negm in f32 PSUM adds; only the smooth
    softmax path sees the fp32r rounding, mask buckets are computed exactly.
  - denominator comes from ones@P^T in fp32r (P in [0,1], err ~2^-21 rel).
  - suffix (fully-masked) columns contribute C = exp(-1e9 - M) per row via
    rank-1 SW x C^T injects and a C * n_suffix denominator fixup (baseline
    mechanism, unchanged).
"""

import numpy as np

import concourse.bass as bass
import concourse.bacc as bacc
import concourse.mybir as mybir
import concourse.tile as tile
from concourse.masks import make_identity

F32 = mybir.dt.float32
F32R = mybir.dt.float32r
BF16 = mybir.dt.bfloat16
ALU = mybir.AluOpType
ACTF = mybir.ActivationFunctionType
AX = mybir.AxisListType

B, NH, T, HD = 1, 4, 4096, 128
STRIDE = 8
TDS = T // STRIDE          # 512 downsampled positions
KDS = 128                  # exact 128th largest per ds row
NEG = -1e9
SCALE = HD ** -0.5
ZAP = -1e30

NB = 16                    # 128-row query blocks per core
QPB = 128
NLQ = NB * QPB             # 2048 local query rows
NG = 4                     # groups of 4 blocks (512 q)
CH = T // 128              # 32 key/value chunks
CMW = 1008                 # sliding ds-causal const width
GORDERS = {0: [0, 1, 2, 3], 1: [0, 1, 2, 3]}
# Same per-program block SETS as the baseline (near-equal causal work), but
# ordered so sds-tile 0 (list positions 0-7 = groups A,B) carries the bulk of
# the PE work: group B's big blocks keep PE busy for the ~19us that tile 1's
# serial top-k chain occupies the DVE.  Groups ascend within themselves
# (the jmin narrowing logic requires it); block 0 of the list must be the
# program's earliest block (desperate-row special handling).
KBS = [
    [0, 2, 4, 6, 24, 27, 29, 31, 8, 10, 12, 14, 18, 20, 22, 23],
    [1, 3, 5, 7, 25, 26, 28, 30, 9, 11, 13, 15, 16, 17, 19, 21],
]


def _consts(nc, pool):
    ident = pool.tile([128, 128], F32, tag="ident")
    make_identity(nc, ident[:])
    identr = pool.tile([128, 128], F32R, tag="identr")
    nc.vector.tensor_copy(identr[:], ident[:])

    # cm3[i, jj] = 0.0 if jj <= 496 + i//8 else -1e9 (sliding ds-causal mask)
    cm3 = pool.tile([128, CMW], F32, tag="cm3")
    nc.gpsimd.memset(cm3[:], 0.0)
    nc.gpsimd.affine_select(
        out=cm3[:], in_=cm3[:], pattern=[[-8, CMW]], base=3968,
        channel_multiplier=1, compare_op=ALU.is_ge, fill=NEG,
    )

    # rep[bp][k, i] = 1.0 iff k == 16*bp + i//8 (f32 for the mask chain,
    # f32r copies for the X^T mask-replication matmuls)
    reps, repsr = [], []
    for bp in range(8):
        rep = pool.tile([128, 128], F32, tag=f"rep{bp}")
        nc.gpsimd.memset(rep[:], 1.0)
        nc.gpsimd.affine_select(
            out=rep[:], in_=rep[:], pattern=[[1, 128]], base=128 * bp,
            channel_multiplier=-8, compare_op=ALU.is_ge, fill=0.0)
        nc.gpsimd.affine_select(
            out=rep[:], in_=rep[:], pattern=[[-1, 128]], base=7 - 128 * bp,
            channel_multiplier=8, compare_op=ALU.is_ge, fill=0.0)
        reps.append(rep)
        repr_ = pool.tile([128, 128], F32R, tag=f"repr{bp}")
        nc.vector.tensor_copy(repr_[:], rep[:])
        repsr.append(repr_)

    onesf = pool.tile([128, 1], F32, tag="onesf")
    nc.gpsimd.memset(onesf[:], 1.0)
    onesr = pool.tile([128, 1], F32R, tag="onesr")
    nc.vector.tensor_copy(onesr[:], onesf[:])
    onesb = pool.tile([128, 1], BF16, tag="onesb")
    nc.gpsimd.memset(onesb[:], 1.0)
    onesrow = pool.tile([1, 128], F32, tag="onesrow")
    nc.gpsimd.memset(onesrow[:], 1.0)

    # c01T[i, j] = 1 where i > j (strictly below diagonal in [k, q] layout =
    # above-diagonal in q-major): the region of the special block's diagonal
    # chunk overwritten with the per-row masked constant C.
    c01t = pool.tile([128, 128], mybir.dt.int8, tag="c01t")
    nc.gpsimd.memset(c01t[:], 1)
    nc.gpsimd.affine_select(
        out=c01t[:], in_=c01t[:], pattern=[[-1, 128]], base=-1,
        channel_multiplier=1, compare_op=ALU.is_ge, fill=0,
    )

    return ident, identr, cm3, reps, repsr, onesr, onesb, onesrow, c01t


def _kernel_body(tc, r, Qd, Kd, QDSd, KDSd, Vd, Ud, Od, dbg=None):
    nc = tc.nc
    from contextlib import ExitStack
    with ExitStack() as ctx:
        cpool = ctx.enter_context(tc.tile_pool(name="consts", bufs=1))
        inpool = ctx.enter_context(tc.tile_pool(name="inputs", bufs=1))
        # PSUM budget (8 banks): xps 2 + outt 1 + rsps 1 + maskps 2 + ptps 1
        # + scratch 1
        xpool = ctx.enter_context(tc.tile_pool(name="xps", bufs=2, space="PSUM"))
        outtpool = ctx.enter_context(tc.tile_pool(name="outtps", bufs=1, space="PSUM"))
        rspool = ctx.enter_context(tc.tile_pool(name="rsps", bufs=1, space="PSUM"))
        mpspool = ctx.enter_context(tc.tile_pool(name="maskps", bufs=2, space="PSUM"))
        ptpool = ctx.enter_context(tc.tile_pool(name="ptps", bufs=1, space="PSUM"))
        scrpspool = ctx.enter_context(tc.tile_pool(name="scrps", bufs=1, space="PSUM"))
        sdspool = ctx.enter_context(tc.tile_pool(name="sds", bufs=1))
        scrpool = ctx.enter_context(tc.tile_pool(name="scr", bufs=1))
        epool = ctx.enter_context(tc.tile_pool(name="e", bufs=4))
        fpool = ctx.enter_context(tc.tile_pool(name="f", bufs=5))
        aminpool = ctx.enter_context(tc.tile_pool(name="amin", bufs=3))
        am2pool = ctx.enter_context(tc.tile_pool(name="am2", bufs=8))
        amtpool = ctx.enter_context(tc.tile_pool(name="amt", bufs=7))
        smallpool = ctx.enter_context(tc.tile_pool(name="small", bufs=NB))
        tinypool = ctx.enter_context(tc.tile_pool(name="tiny", bufs=6))
        ptspool = ctx.enter_context(tc.tile_pool(name="pts", bufs=6))
        outtsbpool = ctx.enter_context(tc.tile_pool(name="outtsb", bufs=3))
        swsbpool = ctx.enter_context(tc.tile_pool(name="swsb", bufs=1))

        (ident, identr, cm3, reps, repsr, onesr, onesb, onesrow,
         c01t) = _consts(nc, cpool)

        # one shared PSUM scratch bank; disjoint regions, subtile-dep tracked
        scrps = scrpspool.tile([128, 512], F32, tag="scrps")
        swp = scrps[0:1, 0:128]        # suffix colsum accumulator
        ps1b_r = scrps[0:128, 128:129]  # threshold replicate matmul out
        ctp_r = scrps[0:1, 192:320]     # cee^T transpose out
        rst_r = scrps[0:128, 352:353]   # rowsum^T transpose out
        ngt_r = scrps[0:1, 384:512]     # negm^T transpose out (block 0)

        # ---- loads: ds subsets first (they gate the serial top-k chain) ----
        kt = inpool.tile([128, T], F32R, tag="kt")     # K^T [d, t]
        qts = inpool.tile([128, NLQ], F32R, tag="qts")  # Q^T * scale [d, q]
        kdst = inpool.tile([128, TDS], F32, tag="kdst")
        qdst = inpool.tile([128, TDS // 2], F32, tag="qdst")
        prep = ctx.enter_context(tc.tile_pool(name="prep", bufs=2))
        for dsrc, dsdst, nds in ((KDSd, kdst, TDS), (QDSd, qdst, TDS // 2)):
            natd = prep.tile([128, nds], F32, tag="natd", name="natd")
            nc.sync.dma_start(
                out=natd[:].rearrange("p (c d) -> p c d", d=128),
                in_=dsrc.rearrange("(c p s) d -> p c s d", p=128,
                                   s=STRIDE)[:, :, 0, :],
            )
            n3 = natd[:].rearrange("p (c d) -> p c d", d=128)
            ptd = ptpool.tile([128, 512], F32, tag="ptps", name="ppsd")
            for j in range(nds // 128):
                nc.tensor.transpose(
                    ptd[:, 128 * j:128 * j + 128], n3[:, j, :], ident[:])
            # ACT copy: keeps the DVE queue free so topk0 starts immediately
            nc.scalar.copy(dsdst[:], ptd[:, 0:nds])

        ub = inpool.tile([128, NB], F32, tag="ub")
        nc.sync.dma_start(out=ub[:], in_=Ud[:])
        usc = inpool.tile([128, NB], F32, tag="usc")
        nc.vector.tensor_scalar(usc[:], ub[:], 0.0, 1.0, op0=ALU.max,
                                op1=ALU.min)
        nc.vector.tensor_scalar(usc[:], usc[:], 1.0, None, op0=ALU.add)
        vsb = inpool.tile([128, T], F32R, tag="vsb")   # [t, d] natural

        def v_load():
            # V early: the suffix colsums are the main PE work available
            # under the serial topk0 DVE chain
            nc.sync.dma_start(
                out=vsb[:].rearrange("p (c d) -> p c d", d=128),
                in_=Vd.rearrange("(c p) d -> p c d", p=128),
            )

        def full_prep():
            # K^T: 8 pieces of 512 cols; f32r transposes (1.5 cyc/row)
            s3 = Kd.rearrange("(c p) d -> p c d", p=128)
            for c4 in range(8):
                nat = prep.tile([128, 512], F32R, tag="nat")
                nc.sync.dma_start(
                    out=nat[:].rearrange("p (c d) -> p c d", d=128),
                    in_=s3[:, 4 * c4:4 * c4 + 4, :],
                )
                n3 = nat[:].rearrange("p (c d) -> p c d", d=128)
                pt = ptpool.tile([128, 512], F32R, tag="ptps")
                for j in range(4):
                    nc.tensor.transpose(
                        pt[:, 128 * j:128 * j + 128], n3[:, j, :], identr[:])
                nc.any.tensor_copy(kt[:, 512 * c4:512 * c4 + 512], pt[:])
            # Q^T with the score scale folded into the PSUM->SBUF copy
            s3 = Qd.rearrange("(c p) d -> p c d", p=128)
            for c4 in range(4):
                nat = prep.tile([128, 512], F32R, tag="nat")
                nc.sync.dma_start(
                    out=nat[:].rearrange("p (c d) -> p c d", d=128),
                    in_=s3[:, 4 * c4:4 * c4 + 4, :],
                )
                n3 = nat[:].rearrange("p (c d) -> p c d", d=128)
                pt = ptpool.tile([128, 512], F32R, tag="ptps")
                for j in range(4):
                    nc.tensor.transpose(
                        pt[:, 128 * j:128 * j + 128], n3[:, j, :], identr[:])
                nc.scalar.mul(qts[:, 512 * c4:512 * c4 + 512], pt[:], SCALE)

        # ---- downsampled scores + exact per-row 128th largest ----
        # Split so sds matmuls (PE) can be emitted early while the serial
        # DVE top-k rounds are placed where the DVE queue wants them.
        sds_tiles = {}

        def sds_mm(t):
            sds = sdspool.tile([128, TDS + 1], F32, tag=f"sds{t}",
                               name=f"sds{t}")
            sds_tiles[t] = sds
            ps = mpspool.tile([128, 512], F32, tag="maskps", name=f"sdsps{t}")
            nc.tensor.matmul(ps[:], qdst[:, 128 * t:128 * t + 128], kdst[:])
            nc.scalar.mul(sds[:, 0:TDS], ps[:], SCALE)

        def topk_rounds(t):
            sds = sds_tiles[t]
            scr = scrpool.tile([128, TDS], F32, tag="scr", name=f"scr{t}")
            nc.vector.tensor_copy(scr[:], sds[:, 0:TDS])
            maxsc = scrpool.tile([128, 8], F32, tag="maxsc", name=f"maxsc{t}")
            for rnd in range(KDS // 8):
                nc.vector.max(out=maxsc[:], in_=scr[:])
                if rnd < KDS // 8 - 1:
                    nc.vector.match_replace(
                        out=scr[:], in_to_replace=maxsc[:], in_values=scr[:],
                        imm_value=ZAP,
                    )
            nc.vector.tensor_copy(sds[:, TDS:TDS + 1], maxsc[:, 7:8])
            if dbg is not None:
                nc.sync.dma_start(out=dbg[f"SDS{t}"], in_=sds[:])

        sds_mm(0)
        topk_rounds(0)
        v_load()
        full_prep()

        # ---- suffix V column-sums SW(cb), [1, 128] slices in swall ----
        # bf16 V copy makes the narrow [1,128]-out colsum matmuls 1 cyc/row
        # (fp32r pays a 4x penalty below 256-wide); SW only feeds desperate
        # rows via C = exp(-1e9 - M), so 0.4% bf16 rounding is irrelevant.
        osb = inpool.tile([128, NB * 128], F32, tag="osb")
        vsbb = inpool.tile([128, T], BF16, tag="vsbb")
        nc.gpsimd.tensor_copy(vsbb[:], vsb[:])
        swall = swsbpool.tile([1, NB * 128], BF16, tag="swall")
        call = swsbpool.tile([1, NB * 128], BF16, tag="call")
        negmrow4 = swsbpool.tile([1, 512], F32, tag="negmrow4")
        ceerow0 = swsbpool.tile([1, 128], F32, tag="ceerow0")
        amtf = inpool.tile([128, 512], F32, tag="amtf")   # gA amin^T (f32)
        # cbm[k, q] = C(q) for k > q else 0: the diag chunk's above-diagonal
        # dust of the desperate-capable block, injected via two rank-128
        # matmuls into rsps and outt (pts itself stays affine-zeroed there)
        cbm = inpool.tile([128, 128], F32R, tag="cbm")
        emitted = 0
        prev = CH
        for b in sorted(range(NB), key=lambda b_: KBS[r][b_], reverse=True):
            cb = KBS[r][b] + 1
            for c in range(cb, prev):
                emitted += 1
                nc.tensor.matmul(swp, onesb[:], vsbb[:, 128 * c:128 * c + 128],
                                 start=(emitted == 1), stop=(c == cb),
                                 skip_group_check=(emitted != 1))
            prev = cb
            if cb <= CH - 1:
                nc.scalar.copy(swall[0:1, 128 * b:128 * b + 128], swp)

        # ---- mask chain (q-major, bucket-exact; -> amin2 f32r + cee) ----
        # Emitted as per-block STEP thunks so 8 blocks' chains interleave
        # (pipelining the 12-step cross-engine latency chain) and so steps can
        # be pumped into the chunk pipelines of earlier groups.
        negms, cees, am2s = {}, {}, {}

        def chain_steps(b):
            kb = KBS[r][b]
            nd = 16 * (kb + 1)
            tt, pp = divmod(b, 8)
            st = {}

            def s0():
                sds = sds_tiles[tt]
                st["ps"] = mpspool.tile([128, 512], F32, tag="maskps", name="chps")
                nc.tensor.matmul(st["ps"][:, 0:nd], reps[pp][:], sds[:, 0:nd])
                nc.tensor.matmul(ps1b_r, reps[pp][:], sds[:, TDS:TDS + 1])
                st["thru"] = tinypool.tile([128, 1], F32, tag="thru", name="thru")
                nc.vector.tensor_scalar(st["thru"][:], ps1b_r, usc[:, b:b + 1],
                                        None, op0=ALU.mult)

            def s1():
                # m = 1/(1 + exp(-10*arg)); bucket-exact vs XLA f32 sigmoid
                st["arg"] = epool.tile([128, TDS], F32, tag="arg", name="arg")
                nc.vector.tensor_scalar(st["arg"][:, 0:nd], st["ps"][:, 0:nd],
                                        usc[:, b:b + 1], st["thru"][:],
                                        op0=ALU.mult, op1=ALU.subtract)

            def s2():
                st["z"] = epool.tile([128, TDS], F32, tag="z", name="z")
                nc.scalar.activation(st["z"][:, 0:nd], st["arg"][:, 0:nd],
                                     ACTF.Exp, scale=-10.0)

            def s3():
                nc.vector.tensor_scalar(st["z"][:, 0:nd], st["z"][:, 0:nd],
                                        1.0, None, op0=ALU.add)

            def s4():
                st["e"] = epool.tile([128, TDS], F32, tag="e", name="e")
                nc.vector.reciprocal(st["e"][:, 0:nd], st["z"][:, 0:nd])

            def s5():
                st["f"] = fpool.tile([128, TDS], F32, tag="f", name="f")
                nc.gpsimd.tensor_scalar(st["f"][:, 0:nd], st["e"][:, 0:nd],
                                        -1.0, 1.0, op0=ALU.mult, op1=ALU.add)

            def s6():
                nc.gpsimd.tensor_scalar(st["f"][:, 0:nd], st["f"][:, 0:nd],
                                        NEG, None, op0=ALU.mult)

            def s7():
                # amin = min(F, ds-causal): F <= 0 always and cm3 is 0 on all
                # fully-allowed columns, so the min only matters on the last
                # 16 ds cols (the diagonal ds band, cm3[:, 496:512]) -- apply
                # it in place on f, 16 cols instead of nd.
                nc.vector.tensor_tensor(st["f"][:, nd - 16:nd],
                                        st["f"][:, nd - 16:nd],
                                        cm3[:, 496:512], op=ALU.min)
                st["amin"] = st["f"]

            def s8():
                negm = smallpool.tile([128, 1], F32, tag="negm")
                nc.vector.tensor_reduce(negm[:], st["amin"][:, 0:nd], axis=AX.X,
                                        op=ALU.max, negate=True)
                negms[b] = negm

            def s9():
                if b < 4:
                    # group A blocks can hold desperate or SEMI-desperate
                    # rows (|max F| large): the reference's fl(S + F)
                    # quantizes S at ulp(F), so F must stay UNFOLDED in
                    # exact f32 and negm must be added only after the S+F
                    # rounding (fl(fl(S+F) + negm), matching the ref)
                    amu = am2pool.tile([128, TDS], F32, tag="am2", name="amu")
                    nc.vector.tensor_copy(amu[:, 0:nd], st["amin"][:, 0:nd])
                    am2s[b] = amu
                    return
                # amin2 = amin + negm (X^T = S + amin2 needs no later bias);
                # safe here: every masked column of these blocks has C == 0
                am2 = am2pool.tile([128, TDS], F32R, tag="am2", name="am2")
                nc.gpsimd.tensor_scalar(am2[:, 0:nd], st["amin"][:, 0:nd],
                                        negms[b][:], None, op0=ALU.add)
                am2s[b] = am2

            def s10():
                b2 = tinypool.tile([128, 1], F32, tag="b2")
                nc.vector.tensor_scalar(b2[:], negms[b][:], NEG, None,
                                        op0=ALU.add)
                cee = smallpool.tile([128, 1], F32, tag="cee")
                nc.scalar.activation(cee[:], b2[:], ACTF.Exp)
                cees[b] = cee

            def s11():
                kb_ = KBS[r][b]
                if kb_ + 1 <= CH - 1:
                    nc.tensor.transpose(ctp_r, cees[b][:], ident[:])
                    nc.scalar.copy(call[0:1, 128 * b:128 * b + 128], ctp_r)
                if b < 4:
                    # f32 rows for the exact rank-1 negm inject (+ C bcast)
                    if b == 0:
                        nc.scalar.copy(ceerow0[:], ctp_r)
                    nc.tensor.transpose(ngt_r, negms[b][:], ident[:])
                    nc.scalar.copy(negmrow4[0:1, 128 * b:128 * b + 128],
                                   ngt_r)
                if dbg is not None and b == 0:
                    nc.sync.dma_start(out=dbg["E0"], in_=st["e"][:])
                    nc.sync.dma_start(out=dbg["AM0"], in_=st["amin"][:])

            return [s0, s1, s2, s3, s4, s5, s6, s7, s8, s9, s10, s11]

        def interleave_chains(blocks):
            """Wavefront-interleaved chain thunks: chain i runs one step
            behind chain i-1, so pool rotation stays alloc-after-consumer
            and every wave packs different engines."""
            chains = [chain_steps(b) for b in blocks]
            out = []
            for w in range(len(chains) + 11):
                for i, ch in enumerate(chains):
                    s = w - i
                    if 0 <= s < 12:
                        out.append(ch[s])
            return out

        def amt_steps(g):
            """amin2T piece-build thunks for group g; returns (thunks, amts)."""
            kbs = [KBS[r][4 * g + j] for j in range(4)]
            cmax = kbs[3]
            npieces = (cmax + 1 + 7) // 8
            amts = []
            thunks = []
            if g == 0:
                # group A: one exact-f32 amin^T tile for all four blocks
                # (no folded pieces) + block 0's masked-C broadcast tile
                def build_exact(j):
                    def run():
                        b = j
                        nd = 16 * (kbs[j] + 1)
                        pt = ptpool.tile([128, 512], F32, tag="ptps",
                                         name="ptspec")
                        nc.tensor.transpose(pt[0:nd, 128 * j:128 * j + 128],
                                            am2s[b][:, 0:nd], ident[:])
                        nc.any.tensor_copy(
                            amtf[0:nd, 128 * j:128 * j + 128],
                            pt[0:nd, 128 * j:128 * j + 128])
                        if j == 0:
                            nc.tensor.matmul(pt[:, 128:256], onesrow[:],
                                             ceerow0[:])
                            nc.vector.tensor_copy(cbm[:], pt[:, 128:256])
                            nc.gpsimd.affine_select(
                                out=cbm[:], in_=cbm[:], pattern=[[-1, 128]],
                                base=-1, channel_multiplier=1,
                                compare_op=ALU.is_ge, fill=0.0)
                    return run
                for j in range(4):
                    thunks.append(build_exact(j))
                return thunks, amts
            for p in range(npieces):
                amt = amtpool.tile([128, 512], F32R, tag="amt",
                                   name=f"amt{g}_{p}")
                amts.append(amt)

                def build(p=p, amt=amt):
                    pt = ptpool.tile([128, 512], F32R, tag="ptps")
                    for j in range(4):
                        b = 4 * g + j
                        nd = 16 * (kbs[j] + 1)
                        w = min(128, nd - 128 * p)
                        if w <= 0:
                            continue
                        nc.tensor.transpose(
                            pt[0:w, 128 * j:128 * j + 128],
                            am2s[b][:, 128 * p:128 * p + w], identr[:])
                        nc.any.tensor_copy(amt[0:w, 128 * j:128 * j + 128],
                                           pt[0:w, 128 * j:128 * j + 128])
                thunks.append(build)
            return thunks, amts

        # ---- group chunk pipeline ----
        def run_group(g, amts, pump, last=False):
            kbs = [KBS[r][4 * g + j] for j in range(4)]
            cmax = kbs[3]

            outt = outtpool.tile([128, 512], F32, tag="outtps")
            rsps = rspool.tile([1, 512], F32, tag="rsps")

            def emit_score(c):
                jmin = min(j for j in range(4) if kbs[j] >= c)
                lo = 128 * jmin
                klo = 16 * (c % 8)
                xps = xpool.tile([128, 512], F32, tag="xps")
                nc.tensor.matmul(xps[:, lo:512], kt[:, 128 * c:128 * c + 128],
                                 qts[:, 512 * g + lo:512 * g + 512],
                                 start=True, stop=False)
                if g == 0:
                    # group A: exact-f32 A-rep then exact rank-1 negm, so
                    # fl(fl(S + F) + negm) matches the reference bit-level
                    # quantization (F magnitudes up to 1e9 here quantize S)
                    nc.tensor.matmul(xps[:, lo:512],
                                     reps[c % 8][0:klo + 16, :],
                                     amtf[0:klo + 16, lo:512],
                                     start=False, stop=False,
                                     skip_group_check=True)
                    nc.tensor.matmul(xps[:, lo:512], onesrow[:],
                                     negmrow4[0:1, lo:512],
                                     start=False, stop=True,
                                     skip_group_check=True)
                else:
                    nc.tensor.matmul(xps[:, lo:512],
                                     repsr[c % 8][0:klo + 16, :],
                                     amts[c // 8][0:klo + 16, lo:512],
                                     start=False, stop=True,
                                     skip_group_check=True)
                pts = ptspool.tile([128, 512], F32R, tag="pts")
                nc.scalar.activation(pts[:, lo:512], xps[:, lo:512], ACTF.Exp)
                if c in kbs:
                    j = kbs.index(c)
                    # zero strictly-above-diagonal (k > q); for the special
                    # block the C dust there is re-injected via cbm matmuls
                    nc.gpsimd.affine_select(
                        out=pts[:, 128 * j:128 * j + 128],
                        in_=pts[:, 128 * j:128 * j + 128],
                        pattern=[[1, 128]], base=0, channel_multiplier=-1,
                        compare_op=ALU.is_ge, fill=0.0)
                if dbg is not None and g == 0 and c == 0:
                    nc.sync.dma_start(out=dbg["PTS0"], in_=pts[:])
                return c, lo, pts

            def emit_accum(st):
                # stop whenever some block's columns see their last chunk, so
                # that block's tail (injects/den/epilogue) can read its psum
                # region while the rest keeps accumulating (swp pattern)
                c, lo, pts = st
                stops = (c == cmax) or (last and c in kbs)
                nc.tensor.matmul(rsps[0:1, lo:512], onesr[:], pts[:, lo:512],
                                 start=(c == 0), stop=stops,
                                 skip_group_check=(c != 0))
                nc.tensor.matmul(outt[:, lo:512],
                                 vsb[:, 128 * c:128 * c + 128],
                                 pts[:, lo:512],
                                 start=(c == 0), stop=stops,
                                 skip_group_check=(c != 0))

            def block_tail(j):
                """Emit block j's suffix injects, denominator, O^T->O
                transpose, 1/den scale, and output DMA; valid as soon as
                chunk kbs[j]'s accumulation has stopped."""
                b = 4 * g + j
                nsuf = T - 128 * (kbs[j] + 1)
                if kbs[j] + 1 <= CH - 1:
                    nc.tensor.matmul(
                        outt[:, 128 * j:128 * j + 128],
                        swall[0:1, 128 * b:128 * b + 128],
                        call[0:1, 128 * b:128 * b + 128],
                        start=False, stop=True, skip_group_check=True,
                    )
                if g == 0 and j == 0:
                    # block 0's above-diagonal C dust (desperate rows)
                    dc = kbs[0]
                    nc.tensor.matmul(rsps[0:1, 0:128], onesr[:], cbm[:],
                                     start=False, stop=True,
                                     skip_group_check=True)
                    nc.tensor.matmul(outt[:, 0:128],
                                     vsb[:, 128 * dc:128 * dc + 128], cbm[:],
                                     start=False, stop=True,
                                     skip_group_check=True)
                rs_sb = tinypool.tile([1, 128], F32, tag="rssb")
                nc.scalar.copy(rs_sb[:], rsps[0:1, 128 * j:128 * j + 128])
                nc.tensor.transpose(rst_r, rs_sb[:], ident[0:1, 0:1])
                den = tinypool.tile([128, 1], F32, tag="den")
                if nsuf > 0:
                    nc.vector.scalar_tensor_tensor(
                        out=den[:], in0=cees[b][:], scalar=float(nsuf),
                        in1=rst_r, op0=ALU.mult, op1=ALU.add)
                else:
                    nc.vector.tensor_copy(den[:], rst_r)
                rsum = smallpool.tile([128, 1], F32, tag="rsum")
                nc.vector.reciprocal(rsum[:], den[:])
                outt_sb = outtsbpool.tile([128, 128], F32, tag="outtsb")
                nc.scalar.copy(outt_sb[:], outt[:, 128 * j:128 * j + 128])
                ops = ptpool.tile([128, 512], F32, tag="ptps")
                nc.tensor.transpose(ops[:, 0:128], outt_sb[:], ident[:])
                nc.vector.tensor_scalar(osb[:, 128 * b:128 * b + 128],
                                        ops[:, 0:128], rsum[:], None,
                                        op0=ALU.mult)
                row0 = 128 * b
                nc.sync.dma_start(out=Od[row0:row0 + 128, :],
                                  in_=osb[:, 128 * b:128 * b + 128])

            def group_tail():
                """Whole-group epilogue (non-last groups): fewer, wider ACT
                ops than four block tails."""
                for j in range(4):
                    b = 4 * g + j
                    if kbs[j] + 1 <= CH - 1:
                        nc.tensor.matmul(
                            outt[:, 128 * j:128 * j + 128],
                            swall[0:1, 128 * b:128 * b + 128],
                            call[0:1, 128 * b:128 * b + 128],
                            start=False, stop=True, skip_group_check=True,
                        )
                if g == 0:
                    dc = kbs[0]
                    nc.tensor.matmul(rsps[0:1, 0:128], onesr[:], cbm[:],
                                     start=False, stop=True,
                                     skip_group_check=True)
                    nc.tensor.matmul(outt[:, 0:128],
                                     vsb[:, 128 * dc:128 * dc + 128], cbm[:],
                                     start=False, stop=True,
                                     skip_group_check=True)
                rs_sb = tinypool.tile([1, 512], F32, tag="rssbw")
                nc.scalar.copy(rs_sb[:], rsps[:])
                rsums = []
                for j in range(4):
                    b = 4 * g + j
                    nsuf = T - 128 * (kbs[j] + 1)
                    nc.tensor.transpose(rst_r, rs_sb[0:1, 128 * j:128 * j + 128],
                                        ident[0:1, 0:1])
                    den = tinypool.tile([128, 1], F32, tag="den")
                    if nsuf > 0:
                        nc.vector.scalar_tensor_tensor(
                            out=den[:], in0=cees[b][:], scalar=float(nsuf),
                            in1=rst_r, op0=ALU.mult, op1=ALU.add)
                    else:
                        nc.vector.tensor_copy(den[:], rst_r)
                    rsum = smallpool.tile([128, 1], F32, tag="rsum")
                    nc.vector.reciprocal(rsum[:], den[:])
                    rsums.append(rsum)
                outt_sb = outtsbpool.tile([128, 512], F32, tag="outtsbw")
                nc.scalar.copy(outt_sb[:], outt[:])
                ops = ptpool.tile([128, 512], F32, tag="ptps")
                for j in range(4):
                    nc.tensor.transpose(
                        ops[:, 128 * j:128 * j + 128],
                        outt_sb[:, 128 * j:128 * j + 128], ident[:])
                for j in range(4):
                    b = 4 * g + j
                    nc.vector.tensor_scalar(osb[:, 128 * b:128 * b + 128],
                                            ops[:, 128 * j:128 * j + 128],
                                            rsums[j][:], None, op0=ALU.mult)
                    row0 = 128 * b
                    nc.sync.dma_start(out=Od[row0:row0 + 128, :],
                                      in_=osb[:, 128 * b:128 * b + 128])

            # software-pipelined emission, 2 chunks deep: S/A/exp of chunk c
            # go ahead of rowsum/PV of chunk c-2, so the in-order PE queue
            # has two chunks of slack against ACT exp jitter.  Block tails
            # are emitted as soon as their accumulation stops.  `pump` thunks
            # (later groups' mask chains / amt builds) are spread through the
            # back half of the chunk stream.
            nch = cmax + 1
            pumped = 0
            pend = []

            def after_accum(st):
                # early per-block tails only for the last group (drains the
                # program tail); elsewhere they would steal ACT slots from
                # the exp stream, which is near-critical mid-schedule
                c = st[0]
                if last and c in kbs:
                    block_tail(kbs.index(c))

            for c in range(nch):
                pend.append(emit_score(c))
                if len(pend) > 2:
                    st = pend.pop(0)
                    emit_accum(st)
                    after_accum(st)
                want = (len(pump) * (2 * (c + 1) - nch)) // max(1, nch) \
                    if 2 * (c + 1) > nch else 0
                while pumped < min(want, len(pump)):
                    pump[pumped]()
                    pumped += 1
            for st in pend:
                emit_accum(st)
                after_accum(st)
            if not last:
                group_tail()
            while pumped < len(pump):
                pump[pumped]()
                pumped += 1

        # ---- orchestration ----
        # tile-0 chains pipeline together right after topk0; topk1's serial
        # DVE rounds queue behind them (emitted after the tile-0 chain DVE
        # steps, overlapping the gA/gB pipelines whose tile-0 blocks carry
        # most of the PE work); tile-1 chains + amt builds are pumped into
        # the gB/gC chunk streams.
        gA, gB, gC, gD = GORDERS[r]
        blocksAB = [4 * gA + j for j in range(4)] + [4 * gB + j for j in range(4)]
        sds_mm(1)
        for t in interleave_chains(blocksAB):
            t()
        thA, amtsA = amt_steps(gA)
        for t in thA:
            t()
        topk_rounds(1)
        thB, amtsB = amt_steps(gB)
        run_group(gA, amtsA, pump=thB)
        chainsC = interleave_chains([4 * gC + j for j in range(4)])
        thC, amtsC = amt_steps(gC)
        run_group(gB, amtsB, pump=chainsC + thC)
        chainsD = interleave_chains([4 * gD + j for j in range(4)])
        thD, amtsD = amt_steps(gD)
        run_group(gC, amtsC, pump=chainsD + thD)
        run_group(gD, amtsD, pump=[], last=True)


_PROGRAMS = {}


def build_program(r: int, debug=False):
    key = (r, debug)
    if key in _PROGRAMS:
        return _PROGRAMS[key]
    nc = bacc.Bacc("TRN2", target_bir_lowering=False, debug=False)
    Qd = nc.dram_tensor("Q", [NLQ, HD], F32R, kind="ExternalInput").ap()
    Kd = nc.dram_tensor("K", [T, HD], F32R, kind="ExternalInput").ap()
    QDSd = nc.dram_tensor("QDS", [NLQ, HD], F32, kind="ExternalInput").ap()
    KDSd = nc.dram_tensor("KDS", [T, HD], F32, kind="ExternalInput").ap()
    Vd = nc.dram_tensor("V", [T, HD], F32R, kind="ExternalInput").ap()
    Ud = nc.dram_tensor("UBT", [128, NB], F32, kind="ExternalInput").ap()
    Od = nc.dram_tensor("OUT", [NLQ, HD], F32, kind="ExternalOutput").ap()
    dbg = None
    if debug:
        dbg = {}
        for nm, shp in (("SDS0", [128, TDS + 1]), ("SDS1", [128, TDS + 1]),
                        ("E0", [128, TDS]), ("F0", [128, TDS]),
                        ("AM0", [128, TDS]), ("PTS0", [128, 512])):
            dbg[nm] = nc.dram_tensor(nm, shp, F32, kind="ExternalOutput").ap()
    with tile.TileContext(nc) as tc:
        _kernel_body(tc, r, Qd, Kd, QDSd, KDSd, Vd, Ud, Od, dbg)
    nc.compile()
    _PROGRAMS[key] = nc
    return nc


def shard_inputs(Q, K, V, U):
    """Per-core input dicts: core = 4*r + h (devices 0-3 parity 0)."""
    maps = []
    Qr = Q[0].reshape(NH, 2 * NB, QPB, HD)
    Ur = U[0].reshape(2 * NB, QPB)
    for r in range(2):
        for h in range(NH):
            qsh = np.ascontiguousarray(Qr[h, KBS[r]].reshape(NLQ, HD))
            ubt = np.ascontiguousarray(Ur[KBS[r]].T)
            ksh = np.ascontiguousarray(K[0, h])
            maps.append({
                "Q": qsh,
                "QDS": qsh,
                "K": ksh,
                "KDS": ksh,
                "V": np.ascontiguousarray(V[0, h]),
                "UBT": ubt,
            })
    return maps


def unshard_output(outs):
    O = np.empty((B, NH, T, HD), np.float32)
    Ov = O[0].reshape(NH, 2 * NB, QPB, HD)
    i = 0
    for r in range(2):
        for h in range(NH):
            Ov[h, KBS[r]] = outs[i]["OUT"].reshape(NB, QPB, HD)
            i += 1
    return O


def _run_concurrent(in_maps):
    """Dispatch parity-0 on devices 0-3 and parity-1 on devices 4-7."""
    import jax
    from jax.sharding import Mesh, PartitionSpec
    from jax.experimental.shard_map import shard_map
    from concourse import bass2jax

    bass2jax.install_neuronx_cc_hook()
    devices = jax.devices()
    assert len(devices) >= 8, f"need 8 neuron cores, got {len(devices)}"

    pending = []
    for r in range(2):
        nc = build_program(r)
        maps = in_maps[4 * r:4 * r + 4]
        pname = nc.partition_id_tensor.name if nc.partition_id_tensor else None
        in_names, out_names, out_avals, zero_outs = [], [], [], []
        for alloc in nc.m.functions[0].allocations:
            if not isinstance(alloc, mybir.MemoryLocationSet):
                continue
            name = alloc.memorylocations[0].name
            if alloc.kind == "ExternalInput":
                if name != pname:
                    in_names.append(name)
            elif alloc.kind == "ExternalOutput":
                out_names.append(name)
                shape = tuple(alloc.tensor_shape)
                dtype = mybir.dt.np(alloc.dtype)
                out_avals.append(jax.core.ShapedArray(shape, dtype))
                zero_outs.append(np.zeros(shape, dtype))
        n_params = len(in_names)
        n_outs = len(out_avals)
        all_names = in_names + out_names
        if pname is not None:
            all_names = all_names + [pname]
        donate = tuple(range(n_params, n_params + n_outs))

        def _body(*args, _nc=nc, _avals=tuple(out_avals),
                  _names=tuple(all_names), _onames=tuple(out_names),
                  _pname=pname):
            operands = list(args)
            if _pname is not None:
                operands.append(bass2jax.partition_id_tensor())
            outs = bass2jax._bass_exec_p.bind(
                *operands,
                out_avals=_avals,
                in_names=_names,
                out_names=_onames,
                lowering_input_output_aliases=(),
                sim_require_finite=True,
                sim_require_nnan=True,
                nc=_nc,
            )
            return tuple(outs)

        mesh = Mesh(np.asarray(devices[4 * r:4 * r + 4]), ("core",))
        in_specs = (PartitionSpec("core"),) * (n_params + n_outs)
        out_specs = (PartitionSpec("core"),) * n_outs
        fn = jax.jit(
            shard_map(_body, mesh=mesh, in_specs=in_specs,
                      out_specs=out_specs, check_rep=False),
            donate_argnums=donate, keep_unused=True,
        )
        per_core = [[np.asarray(m[nm]) for nm in in_names] for m in maps]
        concat_in = [
            np.concatenate([per_core[c][i] for c in range(4)], axis=0)
            for i in range(n_params)
        ]
        concat_zero = [
            np.concatenate([z] * 4, axis=0) for z in zero_outs
        ]
        out_arrs = fn(*concat_in, *concat_zero)
        pending.append((out_arrs, out_names))

    results = []
    for r, (out_arrs, out_names) in enumerate(pending):
        outs = [np.asarray(a) for a in out_arrs]
        for c in range(4):
            d = {}
            for i, nm in enumerate(out_names):
                n0 = outs[i].shape[0] // 4
                d[nm] = outs[i][c * n0:(c + 1) * n0]
            results.append(d)
    return results


def kernel(**inputs):
    Q = np.asarray(inputs["Q"], np.float32)
    K = np.asarray(inputs["K"], np.float32)
    V = np.asarray(inputs["V"], np.float32)
    U = np.asarray(inputs["U"], np.float32)
    in_maps = shard_inputs(Q, K, V, U)
    results = _run_concurrent(in_maps)
    return unshard_output(results)
